# revision 7
# baseline (speedup 1.0000x reference)
"""Tensor-parallel GQA attention forward for one TRN2 chip (8 NeuronCores).

Strategy (8-way tensor parallel over heads):
  - each core owns 4 q-heads + 1 kv-head (wq/wk/wv column-sharded, host side)
  - x arrives pre-transposed and pre-cast to bf16 from the host (xT layout
    [128, 16, 256]); RoPE tables arrive pre-replicated; the causal triangle
    mask arrives precomputed
  - projections are sequence-sharded: each core projects its own 256 rows
    against all 3072 weight columns (kv first so its AllToAll is issued
    earliest, then q pair 0, then q pair 1)
  - scores are computed transposed (S^T[k, q]) so exp runs straight out of
    PSUM; softmax denominators come for free as ones-columns in the PV
    matmul; causal masking = skipping k-tiles above the diagonal, a
    column-trapezoid restriction on the 4 diagonal-band tiles (scores, exp
    and PV only touch valid columns), and a 128-wide triangle multiply on
    the diagonal block
  - an AllToAll flips head-sharded attnT to sequence-sharded; the output
    projection's pair-0 half is interleaved into pair-1 attention to keep
    the PE dense, the pair-1 half runs after the final AllToAll
  - compute dtype bf16 (fp32 PSUM accumulation), output fp32
"""

import numpy as np

NC_CORES = 8
SEQ = 2048
DIM = 2048
HD = 64            # head dim
SC = SEQ // NC_CORES   # 256: sequence rows per core (proj shard / output shard)
CH = 512           # q-chunk width for attention
NCH = SEQ // CH    # 4
KT = SEQ // 128    # 16 k-tiles
DT = DIM // 128    # 16 d-tiles

_CACHE = {}


def _build_nc():
    import concourse.bass as bass
    import concourse.mybir as mybir
    import concourse.tile as tile
    from concourse import bacc
    from concourse.masks import make_identity

    BF = mybir.dt.bfloat16
    F32 = mybir.dt.float32
    MUL = mybir.AluOpType.mult
    ADD = mybir.AluOpType.add
    SUB = mybir.AluOpType.subtract

    nc = bacc.Bacc("TRN2", target_bir_lowering=False, debug=False,
                   num_devices=NC_CORES)

    # ---- external I/O (per-core shards) ----
    # W_all columns: [q-pair0: 8x128 | q-pair1: 8x128 | k: 8x64 | v: 8x64]
    xT = nc.dram_tensor("xT", [128, DT, SC], BF, kind="ExternalInput")
    w_all = nc.dram_tensor("w_all", [DIM, DIM + 2 * 512], BF, kind="ExternalInput")
    wo = nc.dram_tensor("wo", [DIM, DIM], BF, kind="ExternalInput")
    cos_rep_in = nc.dram_tensor("cos_rep", [128, 2, 8, 32], BF, kind="ExternalInput")
    sin_rep_in = nc.dram_tensor("sin_rep", [128, 2, 8, 32], BF, kind="ExternalInput")
    tri2_in = nc.dram_tensor("tri2", [128, 2, 128], BF, kind="ExternalInput")
    out = nc.dram_tensor("out", [SC, DIM], F32, kind="ExternalOutput")

    groups = [list(range(NC_CORES))]
    WCOLS = DIM + 1024          # 3072

    with tile.TileContext(nc) as tc:
        # DRAM bounce buffers for collectives
        apkv_in, _ = tc.tile([NC_CORES, SC, 128], BF, space=bass.MemorySpace.DRAM,
                             name="apkv_in")
        apkv_out, _ = tc.tile([NC_CORES, SC, 128], BF, space=bass.MemorySpace.DRAM,
                              addr_space="Shared", name="apkv_out")
        apq0_in, _ = tc.tile([NC_CORES, SC, 128], BF, space=bass.MemorySpace.DRAM,
                             name="apq0_in")
        apq0_out, _ = tc.tile([NC_CORES, SC, 128], BF, space=bass.MemorySpace.DRAM,
                              addr_space="Shared", name="apq0_out")
        apq1_in, _ = tc.tile([NC_CORES, SC, 128], BF, space=bass.MemorySpace.DRAM,
                             name="apq1_in")
        apq1_out, _ = tc.tile([NC_CORES, SC, 128], BF, space=bass.MemorySpace.DRAM,
                              addr_space="Shared", name="apq1_out")
        a2a_in0, _ = tc.tile([NC_CORES, 128, SC], BF,
                             space=bass.MemorySpace.DRAM, name="a2a_in0")
        a2a_out0, _ = tc.tile([NC_CORES, 128, SC], BF,
                              space=bass.MemorySpace.DRAM,
                              addr_space="Shared", name="a2a_out0")
        a2a_in1, _ = tc.tile([NC_CORES, 128, SC], BF,
                             space=bass.MemorySpace.DRAM, name="a2a_in1")
        a2a_out1, _ = tc.tile([NC_CORES, 128, SC], BF,
                              space=bass.MemorySpace.DRAM,
                              addr_space="Shared", name="a2a_out1")

        with tc.tile_pool(name="persist", bufs=1) as pp, \
             tc.tile_pool(name="wstream", bufs=2) as wsp, \
             tc.tile_pool(name="work", bufs=2) as wp, \
             tc.tile_pool(name="psum", bufs=2, space="PSUM") as psp:

            ident = pp.tile([128, 128], BF, name="ident")
            make_identity(nc, ident[:])

            # host-prepped xT slice, RoPE tables, triangle mask
            xTc = pp.tile([128, DT, SC], BF, name="xTc")
            nc.gpsimd.dma_start(xTc[:], xT[:])
            cos_rep = pp.tile([128, 2, 8, 32], BF, name="cos_rep")
            sin_rep = pp.tile([128, 2, 8, 32], BF, name="sin_rep")
            nc.gpsimd.dma_start(cos_rep[:], cos_rep_in[:])
            nc.gpsimd.dma_start(sin_rep[:], sin_rep_in[:])
            tri2 = pp.tile([128, 2, 128], BF, name="tri2")
            nc.gpsimd.dma_start(tri2[:], tri2_in[:])

            # ---------------- seq-sharded projections (all heads, own 256 s) ----
            # W chunk order: k, v first (their A2A is issued earliest), then
            # q-pair0, then q-pair1.
            proj = pp.tile([128, 2, WCOLS], BF, name="proj")

            def proj_chunk(ch):
                wt = wsp.tile([128, DT, CH], BF, tag="wt", bufs=2, name="wt")
                for hf in range(2):
                    nc.sync.dma_start(
                        wt[:, 8 * hf:8 * hf + 8, :],
                        w_all[1024 * hf:1024 * hf + 1024, CH * ch:CH * ch + CH]
                        .rearrange("(t p) m -> p t m", p=128))
                for st in range(2):
                    psq = psp.tile([128, CH], F32, tag="ps", bufs=3, name="psq")
                    for dt in range(DT):
                        nc.tensor.matmul(
                            psq[:], xTc[:, dt, 128 * st:128 * st + 128],
                            wt[:, dt, :],
                            start=(dt == 0), stop=(dt == DT - 1))
                    if ch < 5:   # q and k columns get RoPE (8 head-pairs/chunk)
                        nh = 8
                        pv = psq[:].rearrange("p (h x) -> p h x", x=32)
                        ta = wp.tile([128, 8, 32], F32, tag="ropeA", bufs=2, name="ta")
                        tb = wp.tile([128, 8, 32], F32, tag="ropeB", bufs=2, name="tb")
                        dstv = proj[:, st, CH * ch:CH * ch + CH].rearrange(
                            "p (h x) -> p h x", x=32)
                        crep = cos_rep[:, st, 0:nh, :]
                        srep = sin_rep[:, st, 0:nh, :]
                        qr = pv[:, 0:2 * nh:2, :]
                        qi = pv[:, 1:2 * nh:2, :]
                        nc.vector.tensor_tensor(ta[:, 0:nh, :], qr, crep, MUL)
                        nc.vector.tensor_tensor(tb[:, 0:nh, :], qi, srep, MUL)
                        nc.vector.tensor_tensor(dstv[:, 0:2 * nh:2, :],
                                                ta[:, 0:nh, :], tb[:, 0:nh, :], SUB)
                        nc.vector.tensor_tensor(ta[:, 0:nh, :], qr, srep, MUL)
                        nc.vector.tensor_tensor(tb[:, 0:nh, :], qi, crep, MUL)
                        nc.vector.tensor_tensor(dstv[:, 1:2 * nh:2, :],
                                                ta[:, 0:nh, :], tb[:, 0:nh, :], ADD)
                    else:
                        nc.vector.tensor_copy(proj[:, st, CH * ch:CH * ch + CH],
                                              psq[:])

            # --- kv section ---
            proj_chunk(4)
            for st in range(2):
                nc.gpsimd.dma_start(
                    apkv_in[:, 128 * st:128 * st + 128, 0:64]
                    .rearrange("d p m -> p d m"),
                    proj[:, st, 2048:2560].rearrange("p (d m) -> p d m", m=64))
            proj_chunk(5)
            for st in range(2):
                nc.gpsimd.dma_start(
                    apkv_in[:, 128 * st:128 * st + 128, 64:128]
                    .rearrange("d p m -> p d m"),
                    proj[:, st, 2560:3072].rearrange("p (d m) -> p d m", m=64))
            nc.gpsimd.collective_compute(
                "AllToAll", mybir.AluOpType.bypass,
                replica_groups=groups, ins=[apkv_in.opt()], outs=[apkv_out.opt()],
            )
            # --- q pair 0 ---
            for ch in (0, 1):
                proj_chunk(ch)
                for st in range(2):
                    nc.gpsimd.dma_start(
                        apq0_in[4 * ch:4 * ch + 4, 128 * st:128 * st + 128, :]
                        .rearrange("d p m -> p d m"),
                        proj[:, st, CH * ch:CH * ch + CH]
                        .rearrange("p (d m) -> p d m", m=128))
            nc.gpsimd.collective_compute(
                "AllToAll", mybir.AluOpType.bypass,
                replica_groups=groups, ins=[apq0_in.opt()], outs=[apq0_out.opt()],
            )
            # --- q pair 1 ---
            for ch in (2, 3):
                proj_chunk(ch)
                for st in range(2):
                    nc.gpsimd.dma_start(
                        apq1_in[4 * (ch - 2):4 * (ch - 2) + 4,
                                128 * st:128 * st + 128, :]
                        .rearrange("d p m -> p d m"),
                        proj[:, st, CH * ch:CH * ch + CH]
                        .rearrange("p (d m) -> p d m", m=128))
            nc.gpsimd.collective_compute(
                "AllToAll", mybir.AluOpType.bypass,
                replica_groups=groups, ins=[apq1_in.opt()], outs=[apq1_out.opt()],
            )

            # ---------------- receiver: build kT / v, then qT per pair ----------
            qT_t = [[pp.tile([128, CH], BF, name=f"qT{p}_{j}")
                     for j in range(NCH)] for p in range(2)]
            kT = pp.tile([128, SEQ], BF, name="kT")
            v_sb = pp.tile([128, KT, 2 * HD], BF, name="v_sb")
            nc.gpsimd.memset(v_sb[:, :, HD:2 * HD], 1.0)

            stage_k = pp.tile([128, KT, 64], BF, name="stage_k")
            nc.sync.dma_start(
                stage_k[:],
                apkv_out[:, :, 0:64].rearrange("s (t p) m -> p (s t) m", p=128))
            nc.sync.dma_start(
                v_sb[:, :, 0:HD],
                apkv_out[:, :, 64:128].rearrange("s (t p) m -> p (s t) m", p=128))
            for g in range(KT):
                tk = psp.tile([64, 128], BF, tag="ps", bufs=3, name="tk")
                nc.tensor.transpose(tk[:], stage_k[:, g, :], ident[:])
                nc.vector.tensor_copy(kT[0:64, 128 * g:128 * g + 128], tk[:])
            nc.vector.tensor_copy(kT[64:128, :], kT[0:64, :])

            stage_q = pp.tile([128, 2, KT, 128], BF, name="stage_q")

            def build_qT(pair):
                apq_out = apq0_out if pair == 0 else apq1_out
                nc.sync.dma_start(
                    stage_q[:, pair, :, :],
                    apq_out[:].rearrange("s (t p) m -> p (s t) m", p=128))
                for g in range(KT):
                    tq = psp.tile([128, 128], BF, tag="ps", bufs=3, name="tq")
                    nc.tensor.transpose(tq[:], stage_q[:, pair, g, :], ident[:])
                    nc.vector.tensor_copy(
                        qT_t[pair][g // 4][:, 128 * (g % 4):128 * (g % 4) + 128],
                        tq[:])

            build_qT(0)

            # ---------------- attention ----------------
            attnT = pp.tile([128, 2, SEQ], BF, name="attnT")

            def attention(pair, j, interleave=None):
                nkt = 4 * j + 4
                pso0 = psp.tile([2 * HD, CH], F32, tag="ps", bufs=3, name="pso0")
                pso1 = psp.tile([2 * HD, CH], F32, tag="ps", bufs=3, name="pso1")
                qsl = slice(CH * j, CH * j + CH)
                qTc = qT_t[pair][j]
                for kt in range(nkt):
                    ks = slice(128 * kt, 128 * kt + 128)
                    t = kt - 4 * j        # >= 0 on the diagonal band
                    c0 = 128 * t if t >= 0 else 0
                    sp = psp.tile([128, 2, CH], F32, tag="spair", bufs=2, name="sp")
                    nc.tensor.matmul(sp[:, 0, c0:CH], kT[0:64, ks],
                                     qTc[0:64, c0:CH], start=True, stop=True)
                    nc.tensor.matmul(sp[:, 1, c0:CH], kT[64:128, ks],
                                     qTc[64:128, c0:CH], start=True, stop=True)
                    ep = wp.tile([128, 2, CH], BF, tag="exps", bufs=4, name="ep")
                    nc.scalar.activation(ep[:, :, c0:CH], sp[:, :, c0:CH],
                                         mybir.ActivationFunctionType.Exp,
                                         scale=0.125)
                    if t >= 0:
                        nc.vector.tensor_tensor(ep[:, :, c0:c0 + 128],
                                                ep[:, :, c0:c0 + 128],
                                                tri2[:], MUL)
                    nc.tensor.matmul(pso0[:, c0:CH], v_sb[:, kt, :],
                                     ep[:, 0, c0:CH],
                                     start=(kt == 0), stop=(kt == nkt - 1))
                    nc.tensor.matmul(pso1[:, c0:CH], v_sb[:, kt, :],
                                     ep[:, 1, c0:CH],
                                     start=(kt == 0), stop=(kt == nkt - 1))
                    if interleave is not None:
                        interleave(j, kt)
                for h, pso in ((0, pso0), (1, pso1)):
                    bc = wp.tile([64, CH], F32, tag="bcast", bufs=2, name="bc")
                    nc.vector.tensor_copy(bc[:], pso[HD:2 * HD, :])
                    rc = wp.tile([64, CH], F32, tag="rcp", bufs=2, name="rc")
                    nc.vector.reciprocal_approx_fast(out=rc[:], in_=bc[:])
                    nc.vector.tensor_tensor(
                        attnT[64 * h:64 * h + 64, pair, qsl],
                        pso[0:HD, :], rc[:], MUL)

            # ---------------- output projection helpers ----------------
            woA = pp.tile([128, DT // 2, DIM], BF, name="woA")
            woB = pp.tile([128, DT // 2, DIM], BF, name="woB")
            a2a_sb0 = pp.tile([128, NC_CORES, SC], BF, name="a2a_sb0")
            a2a_sb1 = pp.tile([128, NC_CORES, SC], BF, name="a2a_sb1")
            partials = pp.tile([128, 2 * NCH, CH], BF, tag="proj",
                               name="partials")
            evens = [2 * src for src in range(NC_CORES)]
            odds = [2 * src + 1 for src in range(NC_CORES)]
            chunks = [(qt, nch) for qt in range(2) for nch in range(NCH)]

            def op_mm(psf, qt, nsl, g, start, stop):
                w_ap = (woA[:, g, nsl] if g < DT // 2
                        else woB[:, g - DT // 2, nsl])
                a_ap = (a2a_sb0[:, g // 2, 128 * qt:128 * qt + 128] if g % 2 == 0
                        else a2a_sb1[:, g // 2, 128 * qt:128 * qt + 128])
                nc.tensor.matmul(psf[:], a_ap, w_ap, start=start, stop=stop)

            def even_group(i8):
                qt, nch2 = chunks[i8]
                psf = psp.tile([128, CH], F32, tag="psf", bufs=1, name="psfE")
                nsl = slice(CH * nch2, CH * nch2 + CH)
                for i, g in enumerate(evens):
                    op_mm(psf, qt, nsl, g, i == 0, i == NC_CORES - 1)
                nc.vector.tensor_copy(partials[:, i8, :], psf[:])

            # even-group emission points inside pair-1 attention: after the
            # a2a_out0 data has certainly landed (chunks 2-3), 4 groups each
            ev_sched = {(2, 2): 0, (2, 5): 1, (2, 8): 2, (2, 11): 3,
                        (3, 2): 4, (3, 5): 5, (3, 9): 6, (3, 13): 7}

            def interleave_ev(j, kt):
                i8 = ev_sched.get((j, kt))
                if i8 is not None:
                    even_group(i8)

            # ---------------- pair-0 attention ----------------
            for j in range(NCH):
                attention(0, j)
                if j == 1:
                    build_qT(1)   # overlaps remaining pair-0 attention
                nc.gpsimd.dma_start(
                    a2a_in0[2 * j:2 * j + 2, :, :]
                    .rearrange("d p m -> p d m"),
                    attnT[:, 0, CH * j:CH * j + CH]
                    .rearrange("p (d m) -> p d m", m=SC))
                # anchored wo prefetch (the scheduler hoists dep-free DMAs)
                nc.vector.tensor_copy(woA[0:1, 2 * j, 0:1],
                                      attnT[0:1, 0, CH * j:CH * j + 1])
                nc.sync.dma_start(
                    woA[:, 2 * j:2 * j + 2, :],
                    wo[256 * j:256 * j + 256, :].rearrange("(t p) n -> p t n",
                                                           p=128))
                if j >= 2:   # woB too: needed by the interleaved even groups
                    jb = j - 2
                    nc.vector.tensor_copy(woB[0:1, 4 * jb, 0:1],
                                          attnT[0:1, 0, CH * j:CH * j + 1])
                    nc.sync.dma_start(
                        woB[:, 4 * jb:4 * jb + 4, :],
                        wo[1024 + 512 * jb:1024 + 512 * jb + 512, :]
                        .rearrange("(t p) n -> p t n", p=128))
            nc.gpsimd.collective_compute(
                "AllToAll", mybir.AluOpType.bypass,
                replica_groups=groups, ins=[a2a_in0.opt()], outs=[a2a_out0.opt()],
            )
            nc.sync.dma_start(a2a_sb0[:],
                              a2a_out0[:].rearrange("s p m -> p s m"))

            # ---------------- pair-1 attention + interleaved even outproj ------
            for j in range(NCH):
                attention(1, j, interleave=interleave_ev)
                nc.gpsimd.dma_start(
                    a2a_in1[2 * j:2 * j + 2, :, :]
                    .rearrange("d p m -> p d m"),
                    attnT[:, 1, CH * j:CH * j + CH]
                    .rearrange("p (d m) -> p d m", m=SC))

            # ---------------- final A2A + odd outproj ----------------
            nc.gpsimd.collective_compute(
                "AllToAll", mybir.AluOpType.bypass,
                replica_groups=groups, ins=[a2a_in1.opt()], outs=[a2a_out1.opt()],
            )
            nc.sync.dma_start(a2a_sb1[:],
                              a2a_out1[:].rearrange("s p m -> p s m"))

            for i8, (qt, nch2) in enumerate(chunks):
                psf = psp.tile([128, CH], F32, tag="psf", bufs=1, name="psfO")
                nsl = slice(CH * nch2, CH * nch2 + CH)
                for i, g in enumerate(odds):
                    op_mm(psf, qt, nsl, g, i == 0, i == NC_CORES - 1)
                osb = wp.tile([128, CH], F32, tag="osb", bufs=2, name="osb")
                nc.vector.tensor_tensor(osb[:], psf[:], partials[:, i8, :], ADD)
                nc.sync.dma_start(out[128 * qt:128 * qt + 128, nsl], osb[:])

    nc.finalize()
    return nc


def _get_nc():
    if "nc" not in _CACHE:
        _CACHE["nc"] = _build_nc()
    return _CACHE["nc"]


_PERM = np.concatenate([np.arange(0, HD, 2), np.arange(1, HD, 2)])  # de-interleave


def _shard(inputs):
    import ml_dtypes
    x = np.ascontiguousarray(inputs["x"][0].astype(np.float32))          # [S, D]
    wq, wk, wv = (np.asarray(inputs[k]).astype(np.float32) for k in ("wq", "wk", "wv"))
    wo = np.ascontiguousarray(np.asarray(inputs["wo"]).astype(ml_dtypes.bfloat16))
    cos = np.asarray(inputs["freqs_cos"]).astype(np.float32)
    sin = np.asarray(inputs["freqs_sin"]).astype(np.float32)
    # W_all columns: [q-pair0 (8x128) | q-pair1 (8x128) | k (8x64) | v (8x64)],
    # q/k head-dims de-interleaved ([32 evens | 32 odds] per head)
    wq_p = wq.reshape(DIM, 32, HD)[:, :, _PERM].reshape(DIM, 32, HD)
    wk_p = wk.reshape(DIM, 8, HD)[:, :, _PERM]
    q0 = np.concatenate([wq_p[:, 4 * c:4 * c + 2, :].reshape(DIM, 128)
                         for c in range(NC_CORES)], axis=1)
    q1 = np.concatenate([wq_p[:, 4 * c + 2:4 * c + 4, :].reshape(DIM, 128)
                         for c in range(NC_CORES)], axis=1)
    w_all = np.ascontiguousarray(
        np.concatenate([q0, q1, wk_p.reshape(DIM, 512), wv], axis=1)
        .astype(ml_dtypes.bfloat16))
    # triangle mask for the diagonal 128x128 block (keep col >= row)
    tri = (np.arange(128)[None, :] >= np.arange(128)[:, None]).astype(np.float32)
    tri2 = np.ascontiguousarray(
        np.broadcast_to(tri[:, None, :], (128, 2, 128)).astype(ml_dtypes.bfloat16))
    in_maps = []
    for c in range(NC_CORES):
        xc = x[SC * c:SC * (c + 1), :]                    # [256, 2048]
        # xT layout [128 part, DT, SC]: [p, t, m] = xc[m, 128 t + p]
        xT = np.ascontiguousarray(
            xc.T.reshape(DT, 128, SC).transpose(1, 0, 2).astype(ml_dtypes.bfloat16))
        cs = cos[SC * c:SC * (c + 1), :].reshape(2, 128, 32)
        sn = sin[SC * c:SC * (c + 1), :].reshape(2, 128, 32)
        cos_rep = np.ascontiguousarray(np.broadcast_to(
            cs.transpose(1, 0, 2)[:, :, None, :], (128, 2, 8, 32))
            .astype(ml_dtypes.bfloat16))
        sin_rep = np.ascontiguousarray(np.broadcast_to(
            sn.transpose(1, 0, 2)[:, :, None, :], (128, 2, 8, 32))
            .astype(ml_dtypes.bfloat16))
        in_maps.append({
            "xT": xT,
            "w_all": w_all,
            "wo": wo,
            "cos_rep": cos_rep,
            "sin_rep": sin_rep,
            "tri2": tri2,
        })
    return in_maps


def kernel(**inputs):
    from concourse.bass_utils import run_bass_kernel_spmd

    nc = _get_nc()
    in_maps = _shard(inputs)
    res = run_bass_kernel_spmd(nc, in_maps, core_ids=list(range(NC_CORES)))
    out = np.concatenate([res.results[c]["out"] for c in range(NC_CORES)], axis=0)
    return out[None].astype(np.float32)


# revision 10
# speedup vs baseline: 1.1561x; 1.1561x over previous
"""Tensor-parallel GQA attention forward for one TRN2 chip (8 NeuronCores).

Strategy (8-way tensor parallel over heads):
  - each core owns 4 q-heads + 1 kv-head (wq/wk/wv column-sharded, host side)
  - x arrives pre-transposed and pre-cast to bf16 from the host (xT layout
    [128, 16, 256]); RoPE tables arrive pre-replicated; the causal triangle
    mask arrives precomputed
  - projections are sequence-sharded: each core projects its own 256 rows
    against all 3072 weight columns; k, v and q-pair-0 go out in a single
    merged AllToAll (minimizes the serial collective chain after the entry
    barrier), q-pair-1 in a second one
  - scores are computed transposed (S^T[k, q]) so exp runs straight out of
    PSUM; softmax denominators come for free as ones-columns in the PV
    matmul; causal masking = skipping k-tiles above the diagonal, a
    column-trapezoid restriction on the 4 diagonal-band tiles, and a
    128-wide triangle multiply on the diagonal block
  - receiver-side kT/qT transposes are staged across the pair-0 attention
    chunks (1-2 per k-tile) so the PE stays dense and the first exp starts
    as early as possible
  - an AllToAll flips head-sharded attnT to sequence-sharded; the output
    projection's pair-0 half is drip-fed into pair-1 attention (2 matmuls
    per k-tile) with three groups reserved to fill the final-AllToAll
    window; the pair-1 half runs after it
  - compute dtype bf16 (fp32 PSUM accumulation), output fp32
"""

import numpy as np

NC_CORES = 8
SEQ = 2048
DIM = 2048
HD = 64            # head dim
SC = SEQ // NC_CORES   # 256: sequence rows per core (proj shard / output shard)
CH = 512           # q-chunk width for attention
NCH = SEQ // CH    # 4
KT = SEQ // 128    # 16 k-tiles
DT = DIM // 128    # 16 d-tiles

_CACHE = {}


def _build_nc():
    import concourse.bass as bass
    import concourse.mybir as mybir
    import concourse.tile as tile
    from concourse import bacc
    from concourse.masks import make_identity

    BF = mybir.dt.bfloat16
    F32 = mybir.dt.float32
    MUL = mybir.AluOpType.mult
    ADD = mybir.AluOpType.add
    SUB = mybir.AluOpType.subtract

    nc = bacc.Bacc("TRN2", target_bir_lowering=False, debug=False,
                   num_devices=NC_CORES)

    # ---- external I/O (per-core shards) ----
    # W_all columns: [q-pair0: 8x128 | q-pair1: 8x128 | k: 8x64 | v: 8x64]
    xT = nc.dram_tensor("xT", [128, DT, SC], BF, kind="ExternalInput")
    w_all = nc.dram_tensor("w_all", [DIM, DIM + 2 * 512], BF, kind="ExternalInput")
    wo = nc.dram_tensor("wo", [DIM, DIM], BF, kind="ExternalInput")
    cos_rep_in = nc.dram_tensor("cos_rep", [128, 2, 8, 32], BF, kind="ExternalInput")
    sin_rep_in = nc.dram_tensor("sin_rep", [128, 2, 8, 32], BF, kind="ExternalInput")
    tri2_in = nc.dram_tensor("tri2", [128, 2, 128], BF, kind="ExternalInput")
    out = nc.dram_tensor("out", [SC, DIM], F32, kind="ExternalOutput")

    groups = [list(range(NC_CORES))]
    WCOLS = DIM + 1024          # 3072

    with tile.TileContext(nc) as tc:
        # DRAM bounce buffers for collectives
        # merged kv + q-pair0: cols [k: 64 | v: 64 | q0: 128]
        ap0_in, _ = tc.tile([NC_CORES, SC, 256], BF, space=bass.MemorySpace.DRAM,
                            name="ap0_in")
        ap0_out, _ = tc.tile([NC_CORES, SC, 256], BF, space=bass.MemorySpace.DRAM,
                             addr_space="Shared", name="ap0_out")
        apq1_in, _ = tc.tile([NC_CORES, SC, 128], BF, space=bass.MemorySpace.DRAM,
                             name="apq1_in")
        apq1_out, _ = tc.tile([NC_CORES, SC, 128], BF, space=bass.MemorySpace.DRAM,
                              addr_space="Shared", name="apq1_out")
        a2a_in0, _ = tc.tile([NC_CORES, 128, SC], BF,
                             space=bass.MemorySpace.DRAM, name="a2a_in0")
        a2a_out0, _ = tc.tile([NC_CORES, 128, SC], BF,
                              space=bass.MemorySpace.DRAM,
                              addr_space="Shared", name="a2a_out0")
        a2a_in1, _ = tc.tile([NC_CORES, 128, SC], BF,
                             space=bass.MemorySpace.DRAM, name="a2a_in1")
        a2a_out1, _ = tc.tile([NC_CORES, 128, SC], BF,
                              space=bass.MemorySpace.DRAM,
                              addr_space="Shared", name="a2a_out1")

        with tc.tile_pool(name="persist", bufs=1) as pp, \
             tc.tile_pool(name="wstream", bufs=2) as wsp, \
             tc.tile_pool(name="work", bufs=2) as wp, \
             tc.tile_pool(name="psum", bufs=2, space="PSUM") as psp:

            ident = pp.tile([128, 128], BF, name="ident")
            make_identity(nc, ident[:])

            # host-prepped xT slice, RoPE tables, triangle mask
            xTc = pp.tile([128, DT, SC], BF, name="xTc")
            nc.sync.dma_start(xTc[:], xT[:])
            cos_rep = pp.tile([128, 2, 8, 32], BF, name="cos_rep")
            sin_rep = pp.tile([128, 2, 8, 32], BF, name="sin_rep")
            nc.gpsimd.dma_start(cos_rep[:], cos_rep_in[:])
            nc.gpsimd.dma_start(sin_rep[:], sin_rep_in[:])
            tri2 = pp.tile([128, 2, 128], BF, name="tri2")
            nc.gpsimd.dma_start(tri2[:], tri2_in[:])

            # ---------------- seq-sharded projections (all heads, own 256 s) ----
            # W chunk order: k, v, q-pair0 first (merged A2A issued earliest),
            # then q-pair1.
            proj = pp.tile([128, 2, WCOLS], BF, name="proj")

            def proj_chunk(ch):
                wt = wsp.tile([128, DT, CH], BF, tag="wt", bufs=2, name="wt")
                for hf in range(2):
                    eng = nc.sync if hf == 0 else nc.scalar
                    eng.dma_start(
                        wt[:, 8 * hf:8 * hf + 8, :],
                        w_all[1024 * hf:1024 * hf + 1024, CH * ch:CH * ch + CH]
                        .rearrange("(t p) m -> p t m", p=128))
                for st in range(2):
                    psq = psp.tile([128, CH], F32, tag="ps", bufs=2, name="psq")
                    for dt in range(DT):
                        nc.tensor.matmul(
                            psq[:], xTc[:, dt, 128 * st:128 * st + 128],
                            wt[:, dt, :],
                            start=(dt == 0), stop=(dt == DT - 1))
                    if ch < 5:   # q and k columns get RoPE (8 head-pairs/chunk)
                        nh = 8
                        pv = psq[:].rearrange("p (h x) -> p h x", x=32)
                        ta = wp.tile([128, 8, 32], F32, tag="ropeA", bufs=2, name="ta")
                        tb = wp.tile([128, 8, 32], F32, tag="ropeB", bufs=2, name="tb")
                        dstv = proj[:, st, CH * ch:CH * ch + CH].rearrange(
                            "p (h x) -> p h x", x=32)
                        crep = cos_rep[:, st, 0:nh, :]
                        srep = sin_rep[:, st, 0:nh, :]
                        qr = pv[:, 0:2 * nh:2, :]
                        qi = pv[:, 1:2 * nh:2, :]
                        nc.vector.tensor_tensor(ta[:, 0:nh, :], qr, crep, MUL)
                        nc.vector.tensor_tensor(tb[:, 0:nh, :], qi, srep, MUL)
                        nc.vector.tensor_tensor(dstv[:, 0:2 * nh:2, :],
                                                ta[:, 0:nh, :], tb[:, 0:nh, :], SUB)
                        nc.vector.tensor_tensor(ta[:, 0:nh, :], qr, srep, MUL)
                        nc.vector.tensor_tensor(tb[:, 0:nh, :], qi, crep, MUL)
                        nc.vector.tensor_tensor(dstv[:, 1:2 * nh:2, :],
                                                ta[:, 0:nh, :], tb[:, 0:nh, :], ADD)
                    else:
                        nc.vector.tensor_copy(proj[:, st, CH * ch:CH * ch + CH],
                                              psq[:])

            # --- k, v, q-pair0 -> merged A2A ---
            proj_chunk(4)
            for st in range(2):
                nc.gpsimd.dma_start(
                    ap0_in[:, 128 * st:128 * st + 128, 0:64]
                    .rearrange("d p m -> p d m"),
                    proj[:, st, 2048:2560].rearrange("p (d m) -> p d m", m=64))
            proj_chunk(5)
            for st in range(2):
                nc.gpsimd.dma_start(
                    ap0_in[:, 128 * st:128 * st + 128, 64:128]
                    .rearrange("d p m -> p d m"),
                    proj[:, st, 2560:3072].rearrange("p (d m) -> p d m", m=64))
            for ch in (0, 1):
                proj_chunk(ch)
                for st in range(2):
                    nc.gpsimd.dma_start(
                        ap0_in[4 * ch:4 * ch + 4, 128 * st:128 * st + 128, 128:256]
                        .rearrange("d p m -> p d m"),
                        proj[:, st, CH * ch:CH * ch + CH]
                        .rearrange("p (d m) -> p d m", m=128))
            nc.gpsimd.collective_compute(
                "AllToAll", mybir.AluOpType.bypass,
                replica_groups=groups, ins=[ap0_in.opt()], outs=[ap0_out.opt()],
            )
            # --- q pair 1 ---
            for ch in (2, 3):
                proj_chunk(ch)
                for st in range(2):
                    nc.gpsimd.dma_start(
                        apq1_in[4 * (ch - 2):4 * (ch - 2) + 4,
                                128 * st:128 * st + 128, :]
                        .rearrange("d p m -> p d m"),
                        proj[:, st, CH * ch:CH * ch + CH]
                        .rearrange("p (d m) -> p d m", m=128))
            nc.gpsimd.collective_compute(
                "AllToAll", mybir.AluOpType.bypass,
                replica_groups=groups, ins=[apq1_in.opt()], outs=[apq1_out.opt()],
            )

            # ---------------- receiver staging ----------------
            qT_t = [[pp.tile([128, CH], BF, name=f"qT{p}_{j}")
                     for j in range(NCH)] for p in range(2)]
            kT = pp.tile([128, SEQ], BF, name="kT")
            v_sb = pp.tile([128, KT, 2 * HD], BF, name="v_sb")
            nc.gpsimd.memset(v_sb[:, :, HD:2 * HD], 1.0)

            stage_k = pp.tile([128, KT, 64], BF, name="stage_k")
            stage_q = pp.tile([128, 2, KT, 128], BF, name="stage_q")
            nc.sync.dma_start(
                stage_k[:],
                ap0_out[:, :, 0:64].rearrange("s (t p) m -> p (s t) m", p=128))
            nc.sync.dma_start(
                v_sb[:, :, 0:HD],
                ap0_out[:, :, 64:128].rearrange("s (t p) m -> p (s t) m", p=128))
            nc.sync.dma_start(
                stage_q[:, 0, :, :],
                ap0_out[:, :, 128:256].rearrange("s (t p) m -> p (s t) m", p=128))

            def tk_build(g):       # one k-tile transpose into kT
                tk = psp.tile([64, 128], BF, tag="tr", bufs=1, name="tk")
                nc.tensor.transpose(tk[:], stage_k[:, g, :], ident[:])
                nc.vector.tensor_copy(kT[0:64, 128 * g:128 * g + 128], tk[:])
                nc.vector.tensor_copy(kT[64:128, 128 * g:128 * g + 128], tk[:])

            def tq_build(pair, g):  # one q-tile transpose into qT_t
                tq = psp.tile([128, 128], BF, tag="tr", bufs=1, name="tq")
                nc.tensor.transpose(tq[:], stage_q[:, pair, g, :], ident[:])
                nc.vector.tensor_copy(
                    qT_t[pair][g // 4][:, 128 * (g % 4):128 * (g % 4) + 128],
                    tq[:])

            # ---------------- attention ----------------
            attnT = pp.tile([128, 2, SEQ], BF, name="attnT")

            def attention(pair, j, interleave=None):
                nkt = 4 * j + 4
                pso0 = psp.tile([2 * HD, CH], F32, tag="ps", bufs=2, name="pso0")
                pso1 = psp.tile([2 * HD, CH], F32, tag="ps", bufs=2, name="pso1")
                qsl = slice(CH * j, CH * j + CH)
                qTc = qT_t[pair][j]
                for kt in range(nkt):
                    ks = slice(128 * kt, 128 * kt + 128)
                    t = kt - 4 * j        # >= 0 on the diagonal band
                    c0 = 128 * t if t >= 0 else 0
                    sp = psp.tile([128, 2, CH], F32, tag="spair", bufs=2, name="sp")
                    nc.tensor.matmul(sp[:, 0, c0:CH], kT[0:64, ks],
                                     qTc[0:64, c0:CH], start=True, stop=True)
                    nc.tensor.matmul(sp[:, 1, c0:CH], kT[64:128, ks],
                                     qTc[64:128, c0:CH], start=True, stop=True)
                    ep = wp.tile([128, 2, CH], BF, tag="exps", bufs=4, name="ep")
                    nc.scalar.activation(ep[:, :, c0:CH], sp[:, :, c0:CH],
                                         mybir.ActivationFunctionType.Exp,
                                         scale=0.125)
                    if t >= 0:
                        nc.vector.tensor_tensor(ep[:, :, c0:c0 + 128],
                                                ep[:, :, c0:c0 + 128],
                                                tri2[:], MUL)
                    nc.tensor.matmul(pso0[:, c0:CH], v_sb[:, kt, :],
                                     ep[:, 0, c0:CH],
                                     start=(kt == 0), stop=(kt == nkt - 1))
                    nc.tensor.matmul(pso1[:, c0:CH], v_sb[:, kt, :],
                                     ep[:, 1, c0:CH],
                                     start=(kt == 0), stop=(kt == nkt - 1))
                    if interleave is not None:
                        interleave(j, kt)
                for h, pso in ((0, pso0), (1, pso1)):
                    bc = wp.tile([64, CH], F32, tag="bcast", bufs=2, name="bc")
                    nc.vector.tensor_copy(bc[:], pso[HD:2 * HD, :])
                    rc = wp.tile([64, CH], F32, tag="rcp", bufs=2, name="rc")
                    nc.vector.reciprocal_approx_fast(out=rc[:], in_=bc[:])
                    nc.vector.tensor_tensor(
                        attnT[64 * h:64 * h + 64, pair, qsl],
                        pso[0:HD, :], rc[:], MUL)

            # ---------------- output projection helpers ----------------
            woA = pp.tile([128, DT // 2, DIM], BF, name="woA")
            woB = pp.tile([128, DT // 2, DIM], BF, name="woB")
            a2a_sb0 = pp.tile([128, NC_CORES, SC], BF, name="a2a_sb0")
            a2a_sb1 = pp.tile([128, NC_CORES, SC], BF, name="a2a_sb1")
            partials = pp.tile([128, 2 * NCH, CH], BF, tag="proj",
                               name="partials")
            evens = [2 * src for src in range(NC_CORES)]
            odds = [2 * src + 1 for src in range(NC_CORES)]
            chunks = [(qt, nch) for qt in range(2) for nch in range(NCH)]

            def op_mm(psf, qt, nsl, g, start, stop):
                w_ap = (woA[:, g, nsl] if g < DT // 2
                        else woB[:, g - DT // 2, nsl])
                a_ap = (a2a_sb0[:, g // 2, 128 * qt:128 * qt + 128] if g % 2 == 0
                        else a2a_sb1[:, g // 2, 128 * qt:128 * qt + 128])
                nc.tensor.matmul(psf[:], a_ap, w_ap, start=start, stop=stop)

            # drip-feed state for the even (pair-0) half of the out-projection
            ev_state = {"psf": None, "n": 0}

            def even_steps(nsteps):
                # emit `nsteps` matmuls of the even-half out-projection,
                # opening/closing psum groups of 8 as needed
                for _ in range(nsteps):
                    n = ev_state["n"]
                    if n >= 64:
                        return
                    i8, i = divmod(n, NC_CORES)
                    qt, nch2 = chunks[i8]
                    if i == 0:
                        ev_state["psf"] = psp.tile([128, CH], F32, tag="psf",
                                                   bufs=1, name="psfE")
                    nsl = slice(CH * nch2, CH * nch2 + CH)
                    op_mm(ev_state["psf"], qt, nsl, evens[i],
                          i == 0, i == NC_CORES - 1)
                    if i == NC_CORES - 1:
                        nc.vector.tensor_copy(partials[:, i8, :],
                                              ev_state["psf"][:])
                    ev_state["n"] = n + 1

            # ---------------- pair-0 attention ----------------
            # staged receiver builds: transposes needed by chunk j+1 are
            # emitted inside / right after chunk j
            for g in range(4):
                tk_build(g)
            for g in range(4):
                tq_build(0, g)

            # interleave schedules for pair-0: (j, kt) -> list of builds
            p0_sched = {}
            for g in range(4, 16):       # tk 4..15: 4 per chunk j=0,1,2
                j = g // 4 - 1
                p0_sched.setdefault((j, g % 4), []).append(("tk", g))
            for g in range(4, 16):       # tq0 4..15
                j = g // 4 - 1
                p0_sched.setdefault((j, g % 4), []).append(("tq0", g))
            for g in range(16):          # tq1 0..15 inside chunks 2-3
                j = 2 + g // 8
                p0_sched.setdefault((j, 2 + (g % 8)), []).append(("tq1", g))

            def interleave_p0(j, kt):
                for kind, g in p0_sched.get((j, kt), ()):
                    if kind == "tk":
                        tk_build(g)
                    elif kind == "tq0":
                        tq_build(0, g)
                    else:
                        tq_build(1, g)

            for j in range(NCH):
                if j == 1:
                    nc.sync.dma_start(
                        stage_q[:, 1, :, :],
                        apq1_out[:].rearrange("s (t p) m -> p (s t) m", p=128))
                attention(0, j, interleave=interleave_p0)
                nc.gpsimd.dma_start(
                    a2a_in0[2 * j:2 * j + 2, :, :]
                    .rearrange("d p m -> p d m"),
                    attnT[:, 0, CH * j:CH * j + CH]
                    .rearrange("p (d m) -> p d m", m=SC))
                # anchored wo prefetch (the scheduler hoists dep-free DMAs)
                nc.vector.tensor_copy(woA[0:1, 2 * j, 0:1],
                                      attnT[0:1, 0, CH * j:CH * j + 1])
                nc.sync.dma_start(
                    woA[:, 2 * j:2 * j + 2, :],
                    wo[256 * j:256 * j + 256, :].rearrange("(t p) n -> p t n",
                                                           p=128))
                if j >= 2:   # woB too: needed by the interleaved even groups
                    jb = j - 2
                    nc.vector.tensor_copy(woB[0:1, 4 * jb, 0:1],
                                          attnT[0:1, 0, CH * j:CH * j + 1])
                    nc.gpsimd.dma_start(
                        woB[:, 4 * jb:4 * jb + 4, :],
                        wo[1024 + 512 * jb:1024 + 512 * jb + 512, :]
                        .rearrange("(t p) n -> p t n", p=128))
            nc.gpsimd.collective_compute(
                "AllToAll", mybir.AluOpType.bypass,
                replica_groups=groups, ins=[a2a_in0.opt()], outs=[a2a_out0.opt()],
            )
            nc.sync.dma_start(a2a_sb0[:],
                              a2a_out0[:].rearrange("s p m -> p s m"))

            # ---------------- pair-1 attention + drip-fed even outproj --------
            # 2 even matmuls per k-tile starting at chunk 1 kt 4; 5 groups
            # (40 matmuls) land inside pair-1, 3 are saved for the final-A2A
            # window
            def interleave_p1(j, kt):
                if (j == 1 and kt >= 6) or j >= 2:
                    if ev_state["n"] < 40:
                        even_steps(2)

            for j in range(NCH):
                attention(1, j, interleave=interleave_p1)
                nc.gpsimd.dma_start(
                    a2a_in1[2 * j:2 * j + 2, :, :]
                    .rearrange("d p m -> p d m"),
                    attnT[:, 1, CH * j:CH * j + CH]
                    .rearrange("p (d m) -> p d m", m=SC))

            # ---------------- final A2A + remaining outproj ----------------
            nc.gpsimd.collective_compute(
                "AllToAll", mybir.AluOpType.bypass,
                replica_groups=groups, ins=[a2a_in1.opt()], outs=[a2a_out1.opt()],
            )
            even_steps(64)        # groups 5-7 fill the A2A window
            nc.sync.dma_start(a2a_sb1[:],
                              a2a_out1[:].rearrange("s p m -> p s m"))

            for i8, (qt, nch2) in enumerate(chunks):
                psf = psp.tile([128, CH], F32, tag="psf", bufs=1, name="psfO")
                nsl = slice(CH * nch2, CH * nch2 + CH)
                for i, g in enumerate(odds):
                    op_mm(psf, qt, nsl, g, i == 0, i == NC_CORES - 1)
                osb = wp.tile([128, CH], F32, tag="osb", bufs=2, name="osb")
                nc.vector.tensor_tensor(osb[:], psf[:], partials[:, i8, :], ADD)
                eng = nc.sync if i8 % 2 == 0 else nc.scalar
                eng.dma_start(out[128 * qt:128 * qt + 128, nsl], osb[:])

    nc.finalize()
    return nc


def _get_nc():
    if "nc" not in _CACHE:
        _CACHE["nc"] = _build_nc()
    return _CACHE["nc"]


_PERM = np.concatenate([np.arange(0, HD, 2), np.arange(1, HD, 2)])  # de-interleave


def _shard(inputs):
    import ml_dtypes
    x = np.ascontiguousarray(inputs["x"][0].astype(np.float32))          # [S, D]
    wq, wk, wv = (np.asarray(inputs[k]).astype(np.float32) for k in ("wq", "wk", "wv"))
    wo = np.ascontiguousarray(np.asarray(inputs["wo"]).astype(ml_dtypes.bfloat16))
    cos = np.asarray(inputs["freqs_cos"]).astype(np.float32)
    sin = np.asarray(inputs["freqs_sin"]).astype(np.float32)
    # W_all columns: [q-pair0 (8x128) | q-pair1 (8x128) | k (8x64) | v (8x64)],
    # q/k head-dims de-interleaved ([32 evens | 32 odds] per head)
    wq_p = wq.reshape(DIM, 32, HD)[:, :, _PERM].reshape(DIM, 32, HD)
    wk_p = wk.reshape(DIM, 8, HD)[:, :, _PERM]
    q0 = np.concatenate([wq_p[:, 4 * c:4 * c + 2, :].reshape(DIM, 128)
                         for c in range(NC_CORES)], axis=1)
    q1 = np.concatenate([wq_p[:, 4 * c + 2:4 * c + 4, :].reshape(DIM, 128)
                         for c in range(NC_CORES)], axis=1)
    w_all = np.ascontiguousarray(
        np.concatenate([q0, q1, wk_p.reshape(DIM, 512), wv], axis=1)
        .astype(ml_dtypes.bfloat16))
    # triangle mask for the diagonal 128x128 block (keep col >= row)
    tri = (np.arange(128)[None, :] >= np.arange(128)[:, None]).astype(np.float32)
    tri2 = np.ascontiguousarray(
        np.broadcast_to(tri[:, None, :], (128, 2, 128)).astype(ml_dtypes.bfloat16))
    in_maps = []
    for c in range(NC_CORES):
        xc = x[SC * c:SC * (c + 1), :]                    # [256, 2048]
        # xT layout [128 part, DT, SC]: [p, t, m] = xc[m, 128 t + p]
        xTl = np.ascontiguousarray(
            xc.T.reshape(DT, 128, SC).transpose(1, 0, 2).astype(ml_dtypes.bfloat16))
        cs = cos[SC * c:SC * (c + 1), :].reshape(2, 128, 32)
        sn = sin[SC * c:SC * (c + 1), :].reshape(2, 128, 32)
        cos_rep = np.ascontiguousarray(np.broadcast_to(
            cs.transpose(1, 0, 2)[:, :, None, :], (128, 2, 8, 32))
            .astype(ml_dtypes.bfloat16))
        sin_rep = np.ascontiguousarray(np.broadcast_to(
            sn.transpose(1, 0, 2)[:, :, None, :], (128, 2, 8, 32))
            .astype(ml_dtypes.bfloat16))
        in_maps.append({
            "xT": xTl,
            "w_all": w_all,
            "wo": wo,
            "cos_rep": cos_rep,
            "sin_rep": sin_rep,
            "tri2": tri2,
        })
    return in_maps


def kernel(**inputs):
    from concourse.bass_utils import run_bass_kernel_spmd

    nc = _get_nc()
    in_maps = _shard(inputs)
    res = run_bass_kernel_spmd(nc, in_maps, core_ids=list(range(NC_CORES)))
    out = np.concatenate([res.results[c]["out"] for c in range(NC_CORES)], axis=0)
    return out[None].astype(np.float32)


# revision 16
# speedup vs baseline: 1.1960x; 1.0346x over previous
"""Tensor-parallel GQA attention forward for one TRN2 chip (8 NeuronCores).

Strategy (8-way tensor parallel over heads):
  - each core owns 4 q-heads + 1 kv-head (wq/wk/wv column-sharded, host side)
  - x arrives pre-transposed and pre-cast to bf16 from the host (xT layout
    [128, 16, 256]); RoPE tables arrive pre-replicated; the causal triangle
    mask arrives precomputed
  - projections are sequence-sharded: each core projects its own 256 rows
    against all 3072 weight columns; k, v and q-pair-0 go out in a single
    merged AllToAll (minimizes the serial collective chain after the entry
    barrier), q-pair-1 in a second one
  - scores are computed transposed (S^T[k, q]) so exp runs straight out of
    PSUM; softmax denominators come for free as ones-columns in the PV
    matmul; causal masking = skipping k-tiles above the diagonal, a
    column-trapezoid restriction on the 4 diagonal-band tiles, and a
    128-wide triangle multiply on the diagonal block
  - receiver-side kT/qT transposes are staged across the pair-0 attention
    chunks (1-2 per k-tile) so the PE stays dense and the first exp starts
    as early as possible
  - an AllToAll flips head-sharded attnT to sequence-sharded; the output
    projection's pair-0 half is drip-fed into pair-1 attention (2 matmuls
    per k-tile) with three groups reserved to fill the final-AllToAll
    window; the pair-1 half runs after it
  - compute dtype bf16 (fp32 PSUM accumulation), output fp32
"""

import numpy as np

NC_CORES = 8
SEQ = 2048
DIM = 2048
HD = 64            # head dim
SC = SEQ // NC_CORES   # 256: sequence rows per core (proj shard / output shard)
CH = 512           # q-chunk width for attention
NCH = SEQ // CH    # 4
KT = SEQ // 128    # 16 k-tiles
DT = DIM // 128    # 16 d-tiles

_CACHE = {}


def _build_nc():
    import concourse.bass as bass
    import concourse.mybir as mybir
    import concourse.tile as tile
    from concourse import bacc
    from concourse.masks import make_identity

    BF = mybir.dt.bfloat16
    F32 = mybir.dt.float32
    MUL = mybir.AluOpType.mult
    ADD = mybir.AluOpType.add
    SUB = mybir.AluOpType.subtract

    nc = bacc.Bacc("TRN2", target_bir_lowering=False, debug=False,
                   num_devices=NC_CORES)

    # ---- external I/O (per-core shards) ----
    # W_all columns: [q-pair0: 8x128 | q-pair1: 8x128 | k: 8x64 | v: 8x64]
    xT = nc.dram_tensor("xT", [128, DT, SC], BF, kind="ExternalInput")
    w_all = nc.dram_tensor("w_all", [DIM, DIM + 2 * 512], BF, kind="ExternalInput")
    wo = nc.dram_tensor("wo", [DIM, DIM], BF, kind="ExternalInput")
    cos_rep_in = nc.dram_tensor("cos_rep", [128, 2, 8, 32], BF, kind="ExternalInput")
    sin_rep_in = nc.dram_tensor("sin_rep", [128, 2, 8, 32], BF, kind="ExternalInput")
    tri2_in = nc.dram_tensor("tri2", [128, 2, 128], BF, kind="ExternalInput")
    out = nc.dram_tensor("out", [SC, DIM], F32, kind="ExternalOutput")

    groups = [list(range(NC_CORES))]
    WCOLS = DIM + 1024          # 3072

    with tile.TileContext(nc) as tc:
        # DRAM bounce buffers for collectives
        apkv_in, _ = tc.tile([NC_CORES, SC, 128], BF, space=bass.MemorySpace.DRAM,
                             name="apkv_in")
        apkv_out, _ = tc.tile([NC_CORES, SC, 128], BF, space=bass.MemorySpace.DRAM,
                              addr_space="Shared", name="apkv_out")
        apq0_in, _ = tc.tile([NC_CORES, SC, 128], BF, space=bass.MemorySpace.DRAM,
                             name="apq0_in")
        apq0_out, _ = tc.tile([NC_CORES, SC, 128], BF, space=bass.MemorySpace.DRAM,
                              addr_space="Shared", name="apq0_out")
        apq1_in, _ = tc.tile([NC_CORES, SC, 128], BF, space=bass.MemorySpace.DRAM,
                             name="apq1_in")
        apq1_out, _ = tc.tile([NC_CORES, SC, 128], BF, space=bass.MemorySpace.DRAM,
                              addr_space="Shared", name="apq1_out")
        a2a_in0, _ = tc.tile([NC_CORES, 128, SC], BF,
                             space=bass.MemorySpace.DRAM, name="a2a_in0")
        a2a_out0, _ = tc.tile([NC_CORES, 128, SC], BF,
                              space=bass.MemorySpace.DRAM,
                              addr_space="Shared", name="a2a_out0")
        a2a_in1, _ = tc.tile([NC_CORES, 128, SC], BF,
                             space=bass.MemorySpace.DRAM, name="a2a_in1")
        a2a_out1, _ = tc.tile([NC_CORES, 128, SC], BF,
                              space=bass.MemorySpace.DRAM,
                              addr_space="Shared", name="a2a_out1")

        with tc.tile_pool(name="persist", bufs=1) as pp, \
             tc.tile_pool(name="wstream", bufs=2) as wsp, \
             tc.tile_pool(name="work", bufs=2) as wp, \
             tc.tile_pool(name="psum", bufs=2, space="PSUM") as psp:

            # host-prepped xT slice first, on its own queue (first proj matmul
            # needs it + the first w chunk; those stream on sync/scalar)
            xTc = pp.tile([128, DT, SC], BF, name="xTc")
            nc.gpsimd.dma_start(xTc[:], xT[:])
            ident = pp.tile([128, 128], BF, name="ident")
            make_identity(nc, ident[:])
            cos_rep = pp.tile([128, 2, 8, 32], BF, name="cos_rep")
            sin_rep = pp.tile([128, 2, 8, 32], BF, name="sin_rep")
            nc.scalar.dma_start(cos_rep[:], cos_rep_in[:])
            nc.scalar.dma_start(sin_rep[:], sin_rep_in[:])
            tri2 = pp.tile([128, 2, 128], BF, name="tri2")
            nc.scalar.dma_start(tri2[:], tri2_in[:])

            # ---------------- seq-sharded projections (all heads, own 256 s) ----
            # W chunk order: k, v, q-pair0 first (merged A2A issued earliest),
            # then q-pair1.
            proj = pp.tile([128, 2, WCOLS], BF, name="proj")

            def proj_chunk(ch):
                wt = wsp.tile([128, DT, CH], BF, tag="wt", bufs=3, name="wt")
                for hf in range(2):
                    eng = nc.sync if hf == 0 else nc.scalar
                    eng.dma_start(
                        wt[:, 8 * hf:8 * hf + 8, :],
                        w_all[1024 * hf:1024 * hf + 1024, CH * ch:CH * ch + CH]
                        .rearrange("(t p) m -> p t m", p=128))
                for st in range(2):
                    psq = psp.tile([128, CH], F32, tag="ps", bufs=2, name="psq")
                    for dt in range(DT):
                        nc.tensor.matmul(
                            psq[:], xTc[:, dt, 128 * st:128 * st + 128],
                            wt[:, dt, :],
                            start=(dt == 0), stop=(dt == DT - 1))
                    if ch < 5:   # q and k columns get RoPE (8 head-pairs/chunk)
                        nh = 8
                        pv = psq[:].rearrange("p (h x) -> p h x", x=32)
                        ta = wp.tile([128, 8, 32], F32, tag="ropeA", bufs=2, name="ta")
                        tb = wp.tile([128, 8, 32], F32, tag="ropeB", bufs=2, name="tb")
                        dstv = proj[:, st, CH * ch:CH * ch + CH].rearrange(
                            "p (h x) -> p h x", x=32)
                        crep = cos_rep[:, st, 0:nh, :]
                        srep = sin_rep[:, st, 0:nh, :]
                        qr = pv[:, 0:2 * nh:2, :]
                        qi = pv[:, 1:2 * nh:2, :]
                        nc.vector.tensor_tensor(ta[:, 0:nh, :], qr, crep, MUL)
                        nc.vector.tensor_tensor(tb[:, 0:nh, :], qi, srep, MUL)
                        nc.vector.tensor_tensor(dstv[:, 0:2 * nh:2, :],
                                                ta[:, 0:nh, :], tb[:, 0:nh, :], SUB)
                        nc.vector.tensor_tensor(ta[:, 0:nh, :], qr, srep, MUL)
                        nc.vector.tensor_tensor(tb[:, 0:nh, :], qi, crep, MUL)
                        nc.vector.tensor_tensor(dstv[:, 1:2 * nh:2, :],
                                                ta[:, 0:nh, :], tb[:, 0:nh, :], ADD)
                    else:
                        nc.vector.tensor_copy(proj[:, st, CH * ch:CH * ch + CH],
                                              psq[:])

            # --- kv -> first A2A (its wire time overlaps the q projections) ---
            proj_chunk(4)
            for st in range(2):
                nc.gpsimd.dma_start(
                    apkv_in[:, 128 * st:128 * st + 128, 0:64]
                    .rearrange("d p m -> p d m"),
                    proj[:, st, 2048:2560].rearrange("p (d m) -> p d m", m=64))
            proj_chunk(5)
            for st in range(2):
                nc.gpsimd.dma_start(
                    apkv_in[:, 128 * st:128 * st + 128, 64:128]
                    .rearrange("d p m -> p d m"),
                    proj[:, st, 2560:3072].rearrange("p (d m) -> p d m", m=64))
            nc.gpsimd.collective_compute(
                "AllToAll", mybir.AluOpType.bypass,
                replica_groups=groups, ins=[apkv_in.opt()], outs=[apkv_out.opt()],
            )
            # --- q pair 0 ---
            for ch in (0, 1):
                proj_chunk(ch)
                for st in range(2):
                    nc.gpsimd.dma_start(
                        apq0_in[4 * ch:4 * ch + 4, 128 * st:128 * st + 128, :]
                        .rearrange("d p m -> p d m"),
                        proj[:, st, CH * ch:CH * ch + CH]
                        .rearrange("p (d m) -> p d m", m=128))
            nc.gpsimd.collective_compute(
                "AllToAll", mybir.AluOpType.bypass,
                replica_groups=groups, ins=[apq0_in.opt()], outs=[apq0_out.opt()],
            )
            # --- q pair 1 ---
            for ch in (2, 3):
                proj_chunk(ch)
                for st in range(2):
                    nc.gpsimd.dma_start(
                        apq1_in[4 * (ch - 2):4 * (ch - 2) + 4,
                                128 * st:128 * st + 128, :]
                        .rearrange("d p m -> p d m"),
                        proj[:, st, CH * ch:CH * ch + CH]
                        .rearrange("p (d m) -> p d m", m=128))
            nc.gpsimd.collective_compute(
                "AllToAll", mybir.AluOpType.bypass,
                replica_groups=groups, ins=[apq1_in.opt()], outs=[apq1_out.opt()],
            )

            # ---------------- receiver staging ----------------
            qT_t = [[pp.tile([128, CH], BF, name=f"qT{p}_{j}")
                     for j in range(NCH)] for p in range(2)]
            kT = pp.tile([128, SEQ], BF, name="kT")
            v_sb = pp.tile([128, KT, 2 * HD], BF, name="v_sb")
            nc.gpsimd.memset(v_sb[:, :, HD:2 * HD], 1.0)

            stage_k = pp.tile([128, KT, 64], BF, name="stage_k")
            stage_q = pp.tile([128, 2, KT, 128], BF, name="stage_q")
            nc.sync.dma_start(
                stage_k[:],
                apkv_out[:, :, 0:64].rearrange("s (t p) m -> p (s t) m", p=128))
            nc.sync.dma_start(
                v_sb[:, :, 0:HD],
                apkv_out[:, :, 64:128].rearrange("s (t p) m -> p (s t) m", p=128))
            nc.sync.dma_start(
                stage_q[:, 0, :, :],
                apq0_out[:].rearrange("s (t p) m -> p (s t) m", p=128))

            def tk_build(g):       # one k-tile transpose into kT
                tk = psp.tile([64, 128], BF, tag="tr", bufs=1, name="tk")
                nc.tensor.transpose(tk[:], stage_k[:, g, :], ident[:])
                nc.vector.tensor_copy(kT[0:64, 128 * g:128 * g + 128], tk[:])
                nc.vector.tensor_copy(kT[64:128, 128 * g:128 * g + 128], tk[:])

            def tq_build(pair, g):  # one q-tile transpose into qT_t
                tq = psp.tile([128, 128], BF, tag="tr", bufs=1, name="tq")
                nc.tensor.transpose(tq[:], stage_q[:, pair, g, :], ident[:])
                nc.vector.tensor_copy(
                    qT_t[pair][g // 4][:, 128 * (g % 4):128 * (g % 4) + 128],
                    tq[:])

            # ---------------- attention ----------------
            attnT = pp.tile([128, 2, SEQ], BF, name="attnT")

            def attention(pair, j, interleave=None):
                nkt = 4 * j + 4
                pso0 = psp.tile([2 * HD, CH], F32, tag="ps", bufs=2, name="pso0")
                pso1 = psp.tile([2 * HD, CH], F32, tag="ps", bufs=2, name="pso1")
                qsl = slice(CH * j, CH * j + CH)
                qTc = qT_t[pair][j]
                for kt in range(nkt):
                    ks = slice(128 * kt, 128 * kt + 128)
                    t = kt - 4 * j        # >= 0 on the diagonal band
                    c0 = 128 * t if t >= 0 else 0
                    sp = psp.tile([128, 2, CH], F32, tag="spair", bufs=2, name="sp")
                    nc.tensor.matmul(sp[:, 0, c0:CH], kT[0:64, ks],
                                     qTc[0:64, c0:CH], start=True, stop=True)
                    nc.tensor.matmul(sp[:, 1, c0:CH], kT[64:128, ks],
                                     qTc[64:128, c0:CH], start=True, stop=True)
                    ep = wp.tile([128, 2, CH], BF, tag="exps", bufs=4, name="ep")
                    nc.scalar.activation(ep[:, :, c0:CH], sp[:, :, c0:CH],
                                         mybir.ActivationFunctionType.Exp,
                                         scale=0.125)
                    if t >= 0:
                        nc.vector.tensor_tensor(ep[:, :, c0:c0 + 128],
                                                ep[:, :, c0:c0 + 128],
                                                tri2[:], MUL)
                    nc.tensor.matmul(pso0[:, c0:CH], v_sb[:, kt, :],
                                     ep[:, 0, c0:CH],
                                     start=(kt == 0), stop=(kt == nkt - 1))
                    nc.tensor.matmul(pso1[:, c0:CH], v_sb[:, kt, :],
                                     ep[:, 1, c0:CH],
                                     start=(kt == 0), stop=(kt == nkt - 1))
                    if interleave is not None:
                        interleave(j, kt)
                for h, pso in ((0, pso0), (1, pso1)):
                    bc = wp.tile([64, CH], F32, tag="bcast", bufs=2, name="bc")
                    nc.vector.tensor_copy(bc[:], pso[HD:2 * HD, :])
                    rc = wp.tile([64, CH], F32, tag="rcp", bufs=2, name="rc")
                    nc.vector.reciprocal_approx_fast(out=rc[:], in_=bc[:])
                    nc.vector.tensor_tensor(
                        attnT[64 * h:64 * h + 64, pair, qsl],
                        pso[0:HD, :], rc[:], MUL)

            # ---------------- output projection helpers ----------------
            woA = pp.tile([128, DT // 2, DIM], BF, name="woA")
            woB = pp.tile([128, DT // 2, DIM], BF, name="woB")
            a2a_sb0 = pp.tile([128, NC_CORES, SC], BF, name="a2a_sb0")
            a2a_sb1 = pp.tile([128, NC_CORES, SC], BF, name="a2a_sb1")
            partials = pp.tile([128, 2 * NCH, CH], BF, tag="proj",
                               name="partials")
            evens = [2 * src for src in range(NC_CORES)]
            odds = [2 * src + 1 for src in range(NC_CORES)]
            chunks = [(qt, nch) for qt in range(2) for nch in range(NCH)]

            def op_mm(psf, qt, nsl, g, start, stop):
                w_ap = (woA[:, g, nsl] if g < DT // 2
                        else woB[:, g - DT // 2, nsl])
                a_ap = (a2a_sb0[:, g // 2, 128 * qt:128 * qt + 128] if g % 2 == 0
                        else a2a_sb1[:, g // 2, 128 * qt:128 * qt + 128])
                nc.tensor.matmul(psf[:], a_ap, w_ap, start=start, stop=stop)

            # drip-feed state for the even (pair-0) half of the out-projection
            ev_state = {"psf": None, "n": 0}

            def even_steps(nsteps):
                # emit `nsteps` matmuls of the even-half out-projection,
                # opening/closing psum groups of 8 as needed
                for _ in range(nsteps):
                    n = ev_state["n"]
                    if n >= 64:
                        return
                    i8, i = divmod(n, NC_CORES)
                    qt, nch2 = chunks[i8]
                    if i == 0:
                        ev_state["psf"] = psp.tile([128, CH], F32, tag="psf",
                                                   bufs=1, name="psfE")
                    nsl = slice(CH * nch2, CH * nch2 + CH)
                    op_mm(ev_state["psf"], qt, nsl, evens[i],
                          i == 0, i == NC_CORES - 1)
                    if i == NC_CORES - 1:
                        nc.vector.tensor_copy(partials[:, i8, :],
                                              ev_state["psf"][:])
                    ev_state["n"] = n + 1

            # ---------------- pair-0 attention ----------------
            # staged receiver builds: transposes needed by chunk j+1 are
            # emitted inside / right after chunk j
            for g in range(4):
                tk_build(g)
            for g in range(4):
                tq_build(0, g)

            # interleave schedules for pair-0: (j, kt) -> list of builds
            p0_sched = {}
            for g in range(4, 16):       # tk 4..15: 4 per chunk j=0,1,2
                j = g // 4 - 1
                p0_sched.setdefault((j, g % 4), []).append(("tk", g))
            for g in range(4, 16):       # tq0 4..15
                j = g // 4 - 1
                p0_sched.setdefault((j, g % 4), []).append(("tq0", g))
            for g in range(16):          # tq1 0..15 inside chunks 2-3
                j = 2 + g // 8
                p0_sched.setdefault((j, 2 + (g % 8)), []).append(("tq1", g))

            def interleave_p0(j, kt):
                for kind, g in p0_sched.get((j, kt), ()):
                    if kind == "tk":
                        tk_build(g)
                    elif kind == "tq0":
                        tq_build(0, g)
                    else:
                        tq_build(1, g)

            for j in range(NCH):
                if j == 1:
                    nc.sync.dma_start(
                        stage_q[:, 1, :, :],
                        apq1_out[:].rearrange("s (t p) m -> p (s t) m", p=128))
                attention(0, j, interleave=interleave_p0)
                nc.gpsimd.dma_start(
                    a2a_in0[2 * j:2 * j + 2, :, :]
                    .rearrange("d p m -> p d m"),
                    attnT[:, 0, CH * j:CH * j + CH]
                    .rearrange("p (d m) -> p d m", m=SC))
                # anchored wo prefetch (the scheduler hoists dep-free DMAs)
                nc.vector.tensor_copy(woA[0:1, 2 * j, 0:1],
                                      attnT[0:1, 0, CH * j:CH * j + 1])
                nc.sync.dma_start(
                    woA[:, 2 * j:2 * j + 2, :],
                    wo[256 * j:256 * j + 256, :].rearrange("(t p) n -> p t n",
                                                           p=128))
                if j >= 2:   # woB too: needed by the interleaved even groups
                    jb = j - 2
                    nc.vector.tensor_copy(woB[0:1, 4 * jb, 0:1],
                                          attnT[0:1, 0, CH * j:CH * j + 1])
                    nc.gpsimd.dma_start(
                        woB[:, 4 * jb:4 * jb + 4, :],
                        wo[1024 + 512 * jb:1024 + 512 * jb + 512, :]
                        .rearrange("(t p) n -> p t n", p=128))
            nc.gpsimd.collective_compute(
                "AllToAll", mybir.AluOpType.bypass,
                replica_groups=groups, ins=[a2a_in0.opt()], outs=[a2a_out0.opt()],
            )
            nc.sync.dma_start(a2a_sb0[:],
                              a2a_out0[:].rearrange("s p m -> p s m"))

            # ---------------- pair-1 attention + drip-fed even outproj --------
            # 2 even matmuls per k-tile starting at chunk 1 kt 4; 5 groups
            # (40 matmuls) land inside pair-1, 3 are saved for the final-A2A
            # window
            def interleave_p1(j, kt):
                if (j == 1 and kt >= 6) or j >= 2:
                    if ev_state["n"] < 40:
                        even_steps(2)

            for j in range(NCH):
                attention(1, j, interleave=interleave_p1)
                nc.gpsimd.dma_start(
                    a2a_in1[2 * j:2 * j + 2, :, :]
                    .rearrange("d p m -> p d m"),
                    attnT[:, 1, CH * j:CH * j + CH]
                    .rearrange("p (d m) -> p d m", m=SC))

            # ---------------- final A2A + remaining outproj ----------------
            nc.gpsimd.collective_compute(
                "AllToAll", mybir.AluOpType.bypass,
                replica_groups=groups, ins=[a2a_in1.opt()], outs=[a2a_out1.opt()],
            )
            even_steps(64)        # groups 5-7 fill the A2A window
            nc.sync.dma_start(a2a_sb1[:],
                              a2a_out1[:].rearrange("s p m -> p s m"))

            for i8, (qt, nch2) in enumerate(chunks):
                psf = psp.tile([128, CH], F32, tag="psf", bufs=1, name="psfO")
                nsl = slice(CH * nch2, CH * nch2 + CH)
                for i, g in enumerate(odds):
                    op_mm(psf, qt, nsl, g, i == 0, i == NC_CORES - 1)
                osb = wp.tile([128, CH], F32, tag="osb", bufs=2, name="osb")
                nc.vector.tensor_tensor(osb[:], psf[:], partials[:, i8, :], ADD)
                eng = nc.sync if i8 % 2 == 0 else nc.scalar
                eng.dma_start(out[128 * qt:128 * qt + 128, nsl], osb[:])

    nc.finalize()
    return nc


def _get_nc():
    if "nc" not in _CACHE:
        _CACHE["nc"] = _build_nc()
    return _CACHE["nc"]


_PERM = np.concatenate([np.arange(0, HD, 2), np.arange(1, HD, 2)])  # de-interleave


def _shard(inputs):
    import ml_dtypes
    x = np.ascontiguousarray(inputs["x"][0].astype(np.float32))          # [S, D]
    wq, wk, wv = (np.asarray(inputs[k]).astype(np.float32) for k in ("wq", "wk", "wv"))
    wo = np.ascontiguousarray(np.asarray(inputs["wo"]).astype(ml_dtypes.bfloat16))
    cos = np.asarray(inputs["freqs_cos"]).astype(np.float32)
    sin = np.asarray(inputs["freqs_sin"]).astype(np.float32)
    # W_all columns: [q-pair0 (8x128) | q-pair1 (8x128) | k (8x64) | v (8x64)],
    # q/k head-dims de-interleaved ([32 evens | 32 odds] per head)
    wq_p = wq.reshape(DIM, 32, HD)[:, :, _PERM].reshape(DIM, 32, HD)
    wk_p = wk.reshape(DIM, 8, HD)[:, :, _PERM]
    q0 = np.concatenate([wq_p[:, 4 * c:4 * c + 2, :].reshape(DIM, 128)
                         for c in range(NC_CORES)], axis=1)
    q1 = np.concatenate([wq_p[:, 4 * c + 2:4 * c + 4, :].reshape(DIM, 128)
                         for c in range(NC_CORES)], axis=1)
    w_all = np.ascontiguousarray(
        np.concatenate([q0, q1, wk_p.reshape(DIM, 512), wv], axis=1)
        .astype(ml_dtypes.bfloat16))
    # triangle mask for the diagonal 128x128 block (keep col >= row)
    tri = (np.arange(128)[None, :] >= np.arange(128)[:, None]).astype(np.float32)
    tri2 = np.ascontiguousarray(
        np.broadcast_to(tri[:, None, :], (128, 2, 128)).astype(ml_dtypes.bfloat16))
    in_maps = []
    for c in range(NC_CORES):
        xc = x[SC * c:SC * (c + 1), :]                    # [256, 2048]
        # xT layout [128 part, DT, SC]: [p, t, m] = xc[m, 128 t + p]
        xTl = np.ascontiguousarray(
            xc.T.reshape(DT, 128, SC).transpose(1, 0, 2).astype(ml_dtypes.bfloat16))
        cs = cos[SC * c:SC * (c + 1), :].reshape(2, 128, 32)
        sn = sin[SC * c:SC * (c + 1), :].reshape(2, 128, 32)
        cos_rep = np.ascontiguousarray(np.broadcast_to(
            cs.transpose(1, 0, 2)[:, :, None, :], (128, 2, 8, 32))
            .astype(ml_dtypes.bfloat16))
        sin_rep = np.ascontiguousarray(np.broadcast_to(
            sn.transpose(1, 0, 2)[:, :, None, :], (128, 2, 8, 32))
            .astype(ml_dtypes.bfloat16))
        in_maps.append({
            "xT": xTl,
            "w_all": w_all,
            "wo": wo,
            "cos_rep": cos_rep,
            "sin_rep": sin_rep,
            "tri2": tri2,
        })
    return in_maps


def kernel(**inputs):
    from concourse.bass_utils import run_bass_kernel_spmd

    nc = _get_nc()
    in_maps = _shard(inputs)
    res = run_bass_kernel_spmd(nc, in_maps, core_ids=list(range(NC_CORES)))
    out = np.concatenate([res.results[c]["out"] for c in range(NC_CORES)], axis=0)
    return out[None].astype(np.float32)


# revision 19
# speedup vs baseline: 1.2514x; 1.0463x over previous
"""Tensor-parallel GQA attention forward for one TRN2 chip (8 NeuronCores).

Strategy (8-way tensor parallel over heads):
  - each core owns 4 q-heads + 1 kv-head (wq/wk/wv column-sharded, host side)
  - x arrives pre-transposed and pre-cast to bf16 from the host (xT layout
    [128, 16, 256]); RoPE tables arrive pre-replicated; the causal triangle
    mask arrives precomputed
  - projections are sequence-sharded: each core projects its own 256 rows
    against all 3072 weight columns; k, v and q-pair-0 go out in a single
    merged AllToAll (minimizes the serial collective chain after the entry
    barrier), q-pair-1 in a second one
  - scores are computed transposed (S^T[k, q]) so exp runs straight out of
    PSUM; softmax denominators come for free as ones-columns in the PV
    matmul; causal masking = skipping k-tiles above the diagonal, a
    column-trapezoid restriction on the 4 diagonal-band tiles, and a
    128-wide triangle multiply on the diagonal block
  - receiver-side kT/qT transposes are staged across the pair-0 attention
    chunks (1-2 per k-tile) so the PE stays dense and the first exp starts
    as early as possible
  - an AllToAll flips head-sharded attnT to sequence-sharded; the output
    projection's pair-0 half is drip-fed into pair-1 attention (2 matmuls
    per k-tile) with three groups reserved to fill the final-AllToAll
    window; the pair-1 half runs after it
  - compute dtype bf16 (fp32 PSUM accumulation), output fp32
"""

import numpy as np

NC_CORES = 8
SEQ = 2048
DIM = 2048
HD = 64            # head dim
SC = SEQ // NC_CORES   # 256: sequence rows per core (proj shard / output shard)
CH = 512           # q-chunk width for attention
NCH = SEQ // CH    # 4
KT = SEQ // 128    # 16 k-tiles
DT = DIM // 128    # 16 d-tiles

_CACHE = {}


def _build_nc():
    import concourse.bass as bass
    import concourse.mybir as mybir
    import concourse.tile as tile
    from concourse import bacc
    from concourse.masks import make_identity

    BF = mybir.dt.bfloat16
    F32 = mybir.dt.float32
    MUL = mybir.AluOpType.mult
    ADD = mybir.AluOpType.add
    SUB = mybir.AluOpType.subtract

    nc = bacc.Bacc("TRN2", target_bir_lowering=False, debug=False,
                   num_devices=NC_CORES)

    # ---- external I/O (per-core shards) ----
    # W_all columns: [q-pair0: 8x128 | q-pair1: 8x128 | k: 8x64 | v: 8x64]
    xT = nc.dram_tensor("xT", [128, DT, SC], BF, kind="ExternalInput")
    w_all = nc.dram_tensor("w_all", [DIM, DIM + 2 * 512], BF, kind="ExternalInput")
    wo = nc.dram_tensor("wo", [DIM, DIM], BF, kind="ExternalInput")
    cos_rep_in = nc.dram_tensor("cos_rep", [128, 2, 8, 32], BF, kind="ExternalInput")
    sin_rep_in = nc.dram_tensor("sin_rep", [128, 2, 8, 32], BF, kind="ExternalInput")
    tri2_in = nc.dram_tensor("tri2", [128, 2, 128], BF, kind="ExternalInput")
    out = nc.dram_tensor("out", [SC, DIM], F32, kind="ExternalOutput")

    groups = [list(range(NC_CORES))]
    WCOLS = DIM + 1024          # 3072

    with tile.TileContext(nc) as tc:
        # DRAM bounce buffers for collectives
        apkv_in, _ = tc.tile([NC_CORES, SC, 128], BF, space=bass.MemorySpace.DRAM,
                             name="apkv_in")
        apkv_out, _ = tc.tile([NC_CORES, SC, 128], BF, space=bass.MemorySpace.DRAM,
                              addr_space="Shared", name="apkv_out")
        apq0_in, _ = tc.tile([NC_CORES, SC, 128], BF, space=bass.MemorySpace.DRAM,
                             name="apq0_in")
        apq0_out, _ = tc.tile([NC_CORES, SC, 128], BF, space=bass.MemorySpace.DRAM,
                              addr_space="Shared", name="apq0_out")
        apq1_in, _ = tc.tile([NC_CORES, SC, 128], BF, space=bass.MemorySpace.DRAM,
                             name="apq1_in")
        apq1_out, _ = tc.tile([NC_CORES, SC, 128], BF, space=bass.MemorySpace.DRAM,
                              addr_space="Shared", name="apq1_out")
        a2a_in0, _ = tc.tile([NC_CORES, 128, SC], BF,
                             space=bass.MemorySpace.DRAM, name="a2a_in0")
        a2a_out0, _ = tc.tile([NC_CORES, 128, SC], BF,
                              space=bass.MemorySpace.DRAM,
                              addr_space="Shared", name="a2a_out0")
        a2a_in1, _ = tc.tile([NC_CORES, 128, SC], BF,
                             space=bass.MemorySpace.DRAM, name="a2a_in1")
        a2a_out1, _ = tc.tile([NC_CORES, 128, SC], BF,
                              space=bass.MemorySpace.DRAM,
                              addr_space="Shared", name="a2a_out1")

        with tc.tile_pool(name="persist", bufs=1) as pp, \
             tc.tile_pool(name="wstream", bufs=2) as wsp, \
             tc.tile_pool(name="work", bufs=2) as wp, \
             tc.tile_pool(name="psum", bufs=2, space="PSUM") as psp:

            # host-prepped xT slice first, on its own queue (first proj matmul
            # needs it + the first w chunk; those stream on sync/scalar)
            xTc = pp.tile([128, DT, SC], BF, name="xTc")
            nc.gpsimd.dma_start(xTc[:], xT[:])
            ident = pp.tile([128, 128], BF, name="ident")
            make_identity(nc, ident[:])
            cos_rep = pp.tile([128, 2, 8, 32], BF, name="cos_rep")
            sin_rep = pp.tile([128, 2, 8, 32], BF, name="sin_rep")
            nc.scalar.dma_start(cos_rep[:], cos_rep_in[:])
            nc.scalar.dma_start(sin_rep[:], sin_rep_in[:])
            tri2 = pp.tile([128, 2, 128], BF, name="tri2")
            nc.scalar.dma_start(tri2[:], tri2_in[:])

            # ---------------- seq-sharded projections (all heads, own 256 s) ----
            # W chunk order: k, v, q-pair0 first (merged A2A issued earliest),
            # then q-pair1.
            proj = pp.tile([128, 2, WCOLS], BF, name="proj")

            def proj_chunk(ch):
                wt = wsp.tile([128, DT, CH], BF, tag="wt", bufs=3, name="wt")
                for hf in range(2):
                    eng = nc.sync if hf == 0 else nc.scalar
                    eng.dma_start(
                        wt[:, 8 * hf:8 * hf + 8, :],
                        w_all[1024 * hf:1024 * hf + 1024, CH * ch:CH * ch + CH]
                        .rearrange("(t p) m -> p t m", p=128))
                for st in range(2):
                    psq = psp.tile([128, CH], F32, tag="ps", bufs=2, name="psq")
                    for dt in range(DT):
                        nc.tensor.matmul(
                            psq[:], xTc[:, dt, 128 * st:128 * st + 128],
                            wt[:, dt, :],
                            start=(dt == 0), stop=(dt == DT - 1))
                    if ch < 5:   # q and k columns get RoPE (8 head-pairs/chunk)
                        nh = 8
                        pv = psq[:].rearrange("p (h x) -> p h x", x=32)
                        ta = wp.tile([128, 8, 32], F32, tag="ropeA", bufs=2, name="ta")
                        tb = wp.tile([128, 8, 32], F32, tag="ropeB", bufs=2, name="tb")
                        dstv = proj[:, st, CH * ch:CH * ch + CH].rearrange(
                            "p (h x) -> p h x", x=32)
                        crep = cos_rep[:, st, 0:nh, :]
                        srep = sin_rep[:, st, 0:nh, :]
                        qr = pv[:, 0:2 * nh:2, :]
                        qi = pv[:, 1:2 * nh:2, :]
                        nc.vector.tensor_tensor(ta[:, 0:nh, :], qr, crep, MUL)
                        nc.vector.tensor_tensor(tb[:, 0:nh, :], qi, srep, MUL)
                        nc.vector.tensor_tensor(dstv[:, 0:2 * nh:2, :],
                                                ta[:, 0:nh, :], tb[:, 0:nh, :], SUB)
                        nc.vector.tensor_tensor(ta[:, 0:nh, :], qr, srep, MUL)
                        nc.vector.tensor_tensor(tb[:, 0:nh, :], qi, crep, MUL)
                        nc.vector.tensor_tensor(dstv[:, 1:2 * nh:2, :],
                                                ta[:, 0:nh, :], tb[:, 0:nh, :], ADD)
                    else:
                        nc.vector.tensor_copy(proj[:, st, CH * ch:CH * ch + CH],
                                              psq[:])

            # --- kv -> first A2A (its wire time overlaps the q projections) ---
            proj_chunk(4)
            for st in range(2):
                nc.gpsimd.dma_start(
                    apkv_in[:, 128 * st:128 * st + 128, 0:64]
                    .rearrange("d p m -> p d m"),
                    proj[:, st, 2048:2560].rearrange("p (d m) -> p d m", m=64))
            proj_chunk(5)
            for st in range(2):
                nc.gpsimd.dma_start(
                    apkv_in[:, 128 * st:128 * st + 128, 64:128]
                    .rearrange("d p m -> p d m"),
                    proj[:, st, 2560:3072].rearrange("p (d m) -> p d m", m=64))
            nc.gpsimd.collective_compute(
                "AllToAll", mybir.AluOpType.bypass,
                replica_groups=groups, ins=[apkv_in.opt()], outs=[apkv_out.opt()],
            )
            # --- q pair 0 ---
            for ch in (0, 1):
                proj_chunk(ch)
                for st in range(2):
                    nc.gpsimd.dma_start(
                        apq0_in[4 * ch:4 * ch + 4, 128 * st:128 * st + 128, :]
                        .rearrange("d p m -> p d m"),
                        proj[:, st, CH * ch:CH * ch + CH]
                        .rearrange("p (d m) -> p d m", m=128))
            nc.gpsimd.collective_compute(
                "AllToAll", mybir.AluOpType.bypass,
                replica_groups=groups, ins=[apq0_in.opt()], outs=[apq0_out.opt()],
            )
            # --- q pair 1 ---
            for ch in (2, 3):
                proj_chunk(ch)
                for st in range(2):
                    nc.gpsimd.dma_start(
                        apq1_in[4 * (ch - 2):4 * (ch - 2) + 4,
                                128 * st:128 * st + 128, :]
                        .rearrange("d p m -> p d m"),
                        proj[:, st, CH * ch:CH * ch + CH]
                        .rearrange("p (d m) -> p d m", m=128))
            nc.gpsimd.collective_compute(
                "AllToAll", mybir.AluOpType.bypass,
                replica_groups=groups, ins=[apq1_in.opt()], outs=[apq1_out.opt()],
            )

            # ---------------- receiver staging ----------------
            qT_t = [[pp.tile([128, CH], BF, name=f"qT{p}_{j}")
                     for j in range(NCH)] for p in range(2)]
            kT = pp.tile([128, SEQ], BF, name="kT")
            v_sb = pp.tile([128, KT, 2 * HD], BF, name="v_sb")
            nc.gpsimd.memset(v_sb[:, :, HD:2 * HD], 1.0)

            stage_k = pp.tile([128, KT, 64], BF, name="stage_k")
            stage_q = pp.tile([128, 2, KT, 128], BF, name="stage_q")
            nc.sync.dma_start(
                stage_k[:],
                apkv_out[:, :, 0:64].rearrange("s (t p) m -> p (s t) m", p=128))
            nc.sync.dma_start(
                v_sb[:, :, 0:HD],
                apkv_out[:, :, 64:128].rearrange("s (t p) m -> p (s t) m", p=128))
            nc.sync.dma_start(
                stage_q[:, 0, :, :],
                apq0_out[:].rearrange("s (t p) m -> p (s t) m", p=128))

            def tk_build(g):       # one k-tile transpose into kT
                tk = psp.tile([64, 128], BF, tag="tr", bufs=1, name="tk")
                nc.tensor.transpose(tk[:], stage_k[:, g, :], ident[:])
                nc.vector.tensor_copy(kT[0:64, 128 * g:128 * g + 128], tk[:])
                nc.vector.tensor_copy(kT[64:128, 128 * g:128 * g + 128], tk[:])

            def tq_build(pair, g):  # one q-tile transpose into qT_t
                tq = psp.tile([128, 128], BF, tag="tr", bufs=1, name="tq")
                nc.tensor.transpose(tq[:], stage_q[:, pair, g, :], ident[:])
                nc.vector.tensor_copy(
                    qT_t[pair][g // 4][:, 128 * (g % 4):128 * (g % 4) + 128],
                    tq[:])

            # ---------------- attention ----------------
            attnT = pp.tile([128, 2, SEQ], BF, name="attnT")

            def attention(pair, j, interleave=None):
                nkt = 4 * j + 4
                pso0 = psp.tile([2 * HD, CH], F32, tag="ps", bufs=2, name="pso0")
                pso1 = psp.tile([2 * HD, CH], F32, tag="ps", bufs=2, name="pso1")
                qsl = slice(CH * j, CH * j + CH)
                qTc = qT_t[pair][j]
                for kt in range(nkt):
                    ks = slice(128 * kt, 128 * kt + 128)
                    t = kt - 4 * j        # >= 0 on the diagonal band
                    c0 = 128 * t if t >= 0 else 0
                    sp = psp.tile([128, 2, CH], F32, tag="spair", bufs=2, name="sp")
                    nc.tensor.matmul(sp[:, 0, c0:CH], kT[0:64, ks],
                                     qTc[0:64, c0:CH], start=True, stop=True)
                    nc.tensor.matmul(sp[:, 1, c0:CH], kT[64:128, ks],
                                     qTc[64:128, c0:CH], start=True, stop=True)
                    ep = wp.tile([128, 2, CH], BF, tag="exps", bufs=4, name="ep")
                    nc.scalar.activation(ep[:, :, c0:CH], sp[:, :, c0:CH],
                                         mybir.ActivationFunctionType.Exp,
                                         scale=0.125)
                    if t >= 0:
                        nc.vector.tensor_tensor(ep[:, :, c0:c0 + 128],
                                                ep[:, :, c0:c0 + 128],
                                                tri2[:], MUL)
                    nc.tensor.matmul(pso0[:, c0:CH], v_sb[:, kt, :],
                                     ep[:, 0, c0:CH],
                                     start=(kt == 0), stop=(kt == nkt - 1))
                    nc.tensor.matmul(pso1[:, c0:CH], v_sb[:, kt, :],
                                     ep[:, 1, c0:CH],
                                     start=(kt == 0), stop=(kt == nkt - 1))
                    if interleave is not None:
                        interleave(j, kt)
                for h, pso in ((0, pso0), (1, pso1)):
                    bc = wp.tile([64, CH], F32, tag="bcast", bufs=2, name="bc")
                    nc.vector.tensor_copy(bc[:], pso[HD:2 * HD, :])
                    rc = wp.tile([64, CH], F32, tag="rcp", bufs=2, name="rc")
                    nc.vector.reciprocal_approx_fast(out=rc[:], in_=bc[:])
                    nc.vector.tensor_tensor(
                        attnT[64 * h:64 * h + 64, pair, qsl],
                        pso[0:HD, :], rc[:], MUL)

            # ---------------- output projection helpers ----------------
            woA = pp.tile([128, DT // 2, DIM], BF, name="woA")
            woB = pp.tile([128, DT // 2, DIM], BF, name="woB")
            a2a_sb0 = pp.tile([128, NC_CORES, SC], BF, name="a2a_sb0")
            a2a_sb1 = pp.tile([128, NC_CORES, SC], BF, name="a2a_sb1")
            partials = pp.tile([128, 2 * NCH, CH], BF, tag="proj",
                               name="partials")
            evens = [2 * src for src in range(NC_CORES)]
            odds = [2 * src + 1 for src in range(NC_CORES)]
            chunks = [(qt, nch) for qt in range(2) for nch in range(NCH)]

            def op_mm(psf, qt, nsl, g, start, stop):
                w_ap = (woA[:, g, nsl] if g < DT // 2
                        else woB[:, g - DT // 2, nsl])
                a_ap = (a2a_sb0[:, g // 2, 128 * qt:128 * qt + 128] if g % 2 == 0
                        else a2a_sb1[:, g // 2, 128 * qt:128 * qt + 128])
                nc.tensor.matmul(psf[:], a_ap, w_ap, start=start, stop=stop)

            # drip-feed state for the even (pair-0) half of the out-projection
            ev_state = {"psf": None, "n": 0}

            def even_steps(nsteps):
                # emit `nsteps` matmuls of the even-half out-projection,
                # opening/closing psum groups of 8 as needed
                for _ in range(nsteps):
                    n = ev_state["n"]
                    if n >= 64:
                        return
                    i8, i = divmod(n, NC_CORES)
                    qt, nch2 = chunks[i8]
                    if i == 0:
                        ev_state["psf"] = psp.tile([128, CH], F32, tag="psf",
                                                   bufs=1, name="psfE")
                    nsl = slice(CH * nch2, CH * nch2 + CH)
                    op_mm(ev_state["psf"], qt, nsl, evens[i],
                          i == 0, i == NC_CORES - 1)
                    if i == NC_CORES - 1:
                        nc.vector.tensor_copy(partials[:, i8, :],
                                              ev_state["psf"][:])
                    ev_state["n"] = n + 1

            # ---------------- pair-0 attention ----------------
            # kT/qT0 builds run upfront: the PE is idle during the A2A wait
            # anyway, and in-attention transposes would stall the exp pipe
            for g in range(KT):
                tk_build(g)
            for g in range(KT):
                tq_build(0, g)

            # pair-1 qT builds drip into pair-0 chunks 2-3, 1 per k-tile
            p0_sched = {}
            for g in range(16):
                j, kt = (2, 1 + g) if g < 11 else (3, g - 11)
                p0_sched[(j, kt)] = g

            def interleave_p0(j, kt):
                g = p0_sched.get((j, kt))
                if g is not None:
                    tq_build(1, g)

            for j in range(NCH):
                if j == 1:
                    nc.sync.dma_start(
                        stage_q[:, 1, :, :],
                        apq1_out[:].rearrange("s (t p) m -> p (s t) m", p=128))
                attention(0, j, interleave=interleave_p0)
                nc.gpsimd.dma_start(
                    a2a_in0[2 * j:2 * j + 2, :, :]
                    .rearrange("d p m -> p d m"),
                    attnT[:, 0, CH * j:CH * j + CH]
                    .rearrange("p (d m) -> p d m", m=SC))
                # anchored wo prefetch (the scheduler hoists dep-free DMAs)
                nc.vector.tensor_copy(woA[0:1, 2 * j, 0:1],
                                      attnT[0:1, 0, CH * j:CH * j + 1])
                nc.sync.dma_start(
                    woA[:, 2 * j:2 * j + 2, :],
                    wo[256 * j:256 * j + 256, :].rearrange("(t p) n -> p t n",
                                                           p=128))
                if j >= 2:   # woB too: needed by the interleaved even groups
                    jb = j - 2
                    nc.vector.tensor_copy(woB[0:1, 4 * jb, 0:1],
                                          attnT[0:1, 0, CH * j:CH * j + 1])
                    nc.gpsimd.dma_start(
                        woB[:, 4 * jb:4 * jb + 4, :],
                        wo[1024 + 512 * jb:1024 + 512 * jb + 512, :]
                        .rearrange("(t p) n -> p t n", p=128))
            nc.gpsimd.collective_compute(
                "AllToAll", mybir.AluOpType.bypass,
                replica_groups=groups, ins=[a2a_in0.opt()], outs=[a2a_out0.opt()],
            )
            for half, eng in ((0, nc.sync), (1, nc.scalar)):
                eng.dma_start(
                    a2a_sb0[:, :, 128 * half:128 * half + 128],
                    a2a_out0[:, :, 128 * half:128 * half + 128]
                    .rearrange("s p m -> p s m"))

            # ---------------- pair-1 attention + drip-fed even outproj --------
            # 1 even matmul per k-tile in chunks 2-3 (3 groups land inside
            # pair-1); 5 groups are saved to fill the final-A2A window
            def interleave_p1(j, kt):
                if j >= 2 and ev_state["n"] < 24:
                    even_steps(1)

            for j in range(NCH):
                attention(1, j, interleave=interleave_p1)
                nc.gpsimd.dma_start(
                    a2a_in1[2 * j:2 * j + 2, :, :]
                    .rearrange("d p m -> p d m"),
                    attnT[:, 1, CH * j:CH * j + CH]
                    .rearrange("p (d m) -> p d m", m=SC))

            # ---------------- final A2A + remaining outproj ----------------
            nc.gpsimd.collective_compute(
                "AllToAll", mybir.AluOpType.bypass,
                replica_groups=groups, ins=[a2a_in1.opt()], outs=[a2a_out1.opt()],
            )
            even_steps(64)        # groups 3-7 fill the A2A window
            for half, eng in ((0, nc.sync), (1, nc.scalar)):
                eng.dma_start(
                    a2a_sb1[:, :, 128 * half:128 * half + 128],
                    a2a_out1[:, :, 128 * half:128 * half + 128]
                    .rearrange("s p m -> p s m"))

            for i8, (qt, nch2) in enumerate(chunks):
                psf = psp.tile([128, CH], F32, tag="psf", bufs=1, name="psfO")
                nsl = slice(CH * nch2, CH * nch2 + CH)
                for i, g in enumerate(odds):
                    op_mm(psf, qt, nsl, g, i == 0, i == NC_CORES - 1)
                osb = wp.tile([128, CH], F32, tag="osb", bufs=2, name="osb")
                nc.vector.tensor_tensor(osb[:], psf[:], partials[:, i8, :], ADD)
                eng = nc.sync if i8 % 2 == 0 else nc.scalar
                eng.dma_start(out[128 * qt:128 * qt + 128, nsl], osb[:])

    nc.finalize()
    return nc


def _get_nc():
    if "nc" not in _CACHE:
        _CACHE["nc"] = _build_nc()
    return _CACHE["nc"]


_PERM = np.concatenate([np.arange(0, HD, 2), np.arange(1, HD, 2)])  # de-interleave


def _shard(inputs):
    import ml_dtypes
    x = np.ascontiguousarray(inputs["x"][0].astype(np.float32))          # [S, D]
    wq, wk, wv = (np.asarray(inputs[k]).astype(np.float32) for k in ("wq", "wk", "wv"))
    wo = np.ascontiguousarray(np.asarray(inputs["wo"]).astype(ml_dtypes.bfloat16))
    cos = np.asarray(inputs["freqs_cos"]).astype(np.float32)
    sin = np.asarray(inputs["freqs_sin"]).astype(np.float32)
    # W_all columns: [q-pair0 (8x128) | q-pair1 (8x128) | k (8x64) | v (8x64)],
    # q/k head-dims de-interleaved ([32 evens | 32 odds] per head)
    wq_p = wq.reshape(DIM, 32, HD)[:, :, _PERM].reshape(DIM, 32, HD)
    wk_p = wk.reshape(DIM, 8, HD)[:, :, _PERM]
    q0 = np.concatenate([wq_p[:, 4 * c:4 * c + 2, :].reshape(DIM, 128)
                         for c in range(NC_CORES)], axis=1)
    q1 = np.concatenate([wq_p[:, 4 * c + 2:4 * c + 4, :].reshape(DIM, 128)
                         for c in range(NC_CORES)], axis=1)
    w_all = np.ascontiguousarray(
        np.concatenate([q0, q1, wk_p.reshape(DIM, 512), wv], axis=1)
        .astype(ml_dtypes.bfloat16))
    # triangle mask for the diagonal 128x128 block (keep col >= row)
    tri = (np.arange(128)[None, :] >= np.arange(128)[:, None]).astype(np.float32)
    tri2 = np.ascontiguousarray(
        np.broadcast_to(tri[:, None, :], (128, 2, 128)).astype(ml_dtypes.bfloat16))
    in_maps = []
    for c in range(NC_CORES):
        xc = x[SC * c:SC * (c + 1), :]                    # [256, 2048]
        # xT layout [128 part, DT, SC]: [p, t, m] = xc[m, 128 t + p]
        xTl = np.ascontiguousarray(
            xc.T.reshape(DT, 128, SC).transpose(1, 0, 2).astype(ml_dtypes.bfloat16))
        cs = cos[SC * c:SC * (c + 1), :].reshape(2, 128, 32)
        sn = sin[SC * c:SC * (c + 1), :].reshape(2, 128, 32)
        cos_rep = np.ascontiguousarray(np.broadcast_to(
            cs.transpose(1, 0, 2)[:, :, None, :], (128, 2, 8, 32))
            .astype(ml_dtypes.bfloat16))
        sin_rep = np.ascontiguousarray(np.broadcast_to(
            sn.transpose(1, 0, 2)[:, :, None, :], (128, 2, 8, 32))
            .astype(ml_dtypes.bfloat16))
        in_maps.append({
            "xT": xTl,
            "w_all": w_all,
            "wo": wo,
            "cos_rep": cos_rep,
            "sin_rep": sin_rep,
            "tri2": tri2,
        })
    return in_maps


def kernel(**inputs):
    from concourse.bass_utils import run_bass_kernel_spmd

    nc = _get_nc()
    in_maps = _shard(inputs)
    res = run_bass_kernel_spmd(nc, in_maps, core_ids=list(range(NC_CORES)))
    out = np.concatenate([res.results[c]["out"] for c in range(NC_CORES)], axis=0)
    return out[None].astype(np.float32)


# revision 28
# speedup vs baseline: 1.3240x; 1.0580x over previous
"""Tensor-parallel GQA attention forward for one TRN2 chip (8 NeuronCores).

Strategy (8-way tensor parallel over heads):
  - each core owns 4 q-heads + 1 kv-head (wq/wk/wv column-sharded, host side)
  - x arrives pre-transposed and pre-cast to bf16 from the host (xT layout
    [128, 16, 256]); RoPE tables arrive pre-replicated; the causal triangle
    mask arrives precomputed
  - projections are sequence-sharded: each core projects its own 256 rows
    against all 3072 weight columns; k, v and q-pair-0 go out in a single
    merged AllToAll (minimizes the serial collective chain after the entry
    barrier), q-pair-1 in a second one
  - scores are computed transposed (S^T[k, q]) so exp runs straight out of
    PSUM; softmax denominators come for free as ones-columns in the PV
    matmul; causal masking = skipping k-tiles above the diagonal, a
    column-trapezoid restriction on the 4 diagonal-band tiles, and a
    128-wide triangle multiply on the diagonal block
  - receiver-side kT/qT transposes are staged across the pair-0 attention
    chunks (1-2 per k-tile) so the PE stays dense and the first exp starts
    as early as possible
  - an AllToAll flips head-sharded attnT to sequence-sharded; the output
    projection's pair-0 half is drip-fed into pair-1 attention (2 matmuls
    per k-tile) with three groups reserved to fill the final-AllToAll
    window; the pair-1 half runs after it
  - compute dtype bf16 (fp32 PSUM accumulation), output fp32
"""

import numpy as np

NC_CORES = 8
SEQ = 2048
DIM = 2048
HD = 64            # head dim
SC = SEQ // NC_CORES   # 256: sequence rows per core (proj shard / output shard)
CH = 512           # q-chunk width for attention
NCH = SEQ // CH    # 4
KT = SEQ // 128    # 16 k-tiles
DT = DIM // 128    # 16 d-tiles

_CACHE = {}


def _build_nc():
    import concourse.bass as bass
    import concourse.mybir as mybir
    import concourse.tile as tile
    from concourse import bacc
    from concourse.masks import make_identity

    BF = mybir.dt.bfloat16
    F32 = mybir.dt.float32
    MUL = mybir.AluOpType.mult
    ADD = mybir.AluOpType.add
    SUB = mybir.AluOpType.subtract

    nc = bacc.Bacc("TRN2", target_bir_lowering=False, debug=False,
                   num_devices=NC_CORES)

    # ---- external I/O (per-core shards) ----
    # W_all columns: [q-pair0: 8x128 | q-pair1: 8x128 | k: 8x64 | v: 8x64]
    xT = nc.dram_tensor("xT", [128, DT, SC], BF, kind="ExternalInput")
    w_all = nc.dram_tensor("w_all", [DIM, DIM + 2 * 512], BF, kind="ExternalInput")
    wo = nc.dram_tensor("wo", [DIM, DIM], BF, kind="ExternalInput")
    cos_rep_in = nc.dram_tensor("cos_rep", [128, 2, 8, 32], BF, kind="ExternalInput")
    sin_rep_in = nc.dram_tensor("sin_rep", [128, 2, 8, 32], BF, kind="ExternalInput")
    tri2_in = nc.dram_tensor("tri2", [128, 2, 128], BF, kind="ExternalInput")
    out = nc.dram_tensor("out", [SC, DIM], F32, kind="ExternalOutput")

    groups = [list(range(NC_CORES))]
    WCOLS = DIM + 1024          # 3072

    with tile.TileContext(nc) as tc:
        # DRAM bounce buffers for collectives
        apkv_in, _ = tc.tile([NC_CORES, SC, 128], BF, space=bass.MemorySpace.DRAM,
                             name="apkv_in")
        apkv_out, _ = tc.tile([NC_CORES, SC, 128], BF, space=bass.MemorySpace.DRAM,
                              addr_space="Shared", name="apkv_out")
        apq0_in, _ = tc.tile([NC_CORES, SC, 128], BF, space=bass.MemorySpace.DRAM,
                             name="apq0_in")
        apq0_out, _ = tc.tile([NC_CORES, SC, 128], BF, space=bass.MemorySpace.DRAM,
                              addr_space="Shared", name="apq0_out")
        apq1_in, _ = tc.tile([NC_CORES, SC, 128], BF, space=bass.MemorySpace.DRAM,
                             name="apq1_in")
        apq1_out, _ = tc.tile([NC_CORES, SC, 128], BF, space=bass.MemorySpace.DRAM,
                              addr_space="Shared", name="apq1_out")
        a2a_in0, _ = tc.tile([NC_CORES, 128, SC], BF,
                             space=bass.MemorySpace.DRAM, name="a2a_in0")
        a2a_out0, _ = tc.tile([NC_CORES, 128, SC], BF,
                              space=bass.MemorySpace.DRAM,
                              addr_space="Shared", name="a2a_out0")
        a2a_in1, _ = tc.tile([NC_CORES, 128, SC], BF,
                             space=bass.MemorySpace.DRAM, name="a2a_in1")
        a2a_out1, _ = tc.tile([NC_CORES, 128, SC], BF,
                              space=bass.MemorySpace.DRAM,
                              addr_space="Shared", name="a2a_out1")

        with tc.tile_pool(name="persist", bufs=1) as pp, \
             tc.tile_pool(name="wstream", bufs=2) as wsp, \
             tc.tile_pool(name="work", bufs=2) as wp, \
             tc.tile_pool(name="psum", bufs=2, space="PSUM") as psp:

            # host-prepped xT slice first, on its own queue (first proj matmul
            # needs it + the first w chunk; those stream on sync/scalar)
            xTc = pp.tile([128, DT, SC], BF, name="xTc")
            nc.gpsimd.dma_start(xTc[:], xT[:])
            ident = pp.tile([128, 128], BF, name="ident")
            make_identity(nc, ident[:])
            cos_rep = pp.tile([128, 2, 8, 32], BF, name="cos_rep")
            sin_rep = pp.tile([128, 2, 8, 32], BF, name="sin_rep")
            nc.scalar.dma_start(cos_rep[:], cos_rep_in[:])
            nc.scalar.dma_start(sin_rep[:], sin_rep_in[:])
            tri2 = pp.tile([128, 2, 128], BF, name="tri2")
            nc.scalar.dma_start(tri2[:], tri2_in[:])

            # ---------------- seq-sharded projections (all heads, own 256 s) ----
            # W chunk order: k, v, q-pair0 first (merged A2A issued earliest),
            # then q-pair1.
            proj = pp.tile([128, 2, WCOLS], BF, name="proj")

            def proj_chunk(ch):
                wt = wsp.tile([128, DT, CH], BF, tag="wt", bufs=3, name="wt")
                for hf in range(2):
                    eng = nc.sync if hf == 0 else nc.scalar
                    eng.dma_start(
                        wt[:, 8 * hf:8 * hf + 8, :],
                        w_all[1024 * hf:1024 * hf + 1024, CH * ch:CH * ch + CH]
                        .rearrange("(t p) m -> p t m", p=128))
                for st in range(2):
                    psq = psp.tile([128, CH], F32, tag="ps", bufs=3, name="psq")
                    for dt in range(DT):
                        nc.tensor.matmul(
                            psq[:], xTc[:, dt, 128 * st:128 * st + 128],
                            wt[:, dt, :],
                            start=(dt == 0), stop=(dt == DT - 1))
                    if ch < 5:   # q and k columns get RoPE (8 head-pairs/chunk)
                        nh = 8
                        pv = psq[:].rearrange("p (h x) -> p h x", x=32)
                        ta = wp.tile([128, 8, 32], F32, tag="ropeA", bufs=2, name="ta")
                        tb = wp.tile([128, 8, 32], F32, tag="ropeB", bufs=2, name="tb")
                        dstv = proj[:, st, CH * ch:CH * ch + CH].rearrange(
                            "p (h x) -> p h x", x=32)
                        crep = cos_rep[:, st, 0:nh, :]
                        srep = sin_rep[:, st, 0:nh, :]
                        qr = pv[:, 0:2 * nh:2, :]
                        qi = pv[:, 1:2 * nh:2, :]
                        nc.vector.tensor_tensor(ta[:, 0:nh, :], qr, crep, MUL)
                        nc.vector.tensor_tensor(tb[:, 0:nh, :], qi, srep, MUL)
                        nc.vector.tensor_tensor(dstv[:, 0:2 * nh:2, :],
                                                ta[:, 0:nh, :], tb[:, 0:nh, :], SUB)
                        nc.vector.tensor_tensor(ta[:, 0:nh, :], qr, srep, MUL)
                        nc.vector.tensor_tensor(tb[:, 0:nh, :], qi, crep, MUL)
                        nc.vector.tensor_tensor(dstv[:, 1:2 * nh:2, :],
                                                ta[:, 0:nh, :], tb[:, 0:nh, :], ADD)
                    else:
                        nc.vector.tensor_copy(proj[:, st, CH * ch:CH * ch + CH],
                                              psq[:])

            # --- kv -> first A2A (its wire time overlaps the q projections) ---
            proj_chunk(4)
            for st in range(2):
                nc.gpsimd.dma_start(
                    apkv_in[:, 128 * st:128 * st + 128, 0:64]
                    .rearrange("d p m -> p d m"),
                    proj[:, st, 2048:2560].rearrange("p (d m) -> p d m", m=64))
            proj_chunk(5)
            for st in range(2):
                nc.gpsimd.dma_start(
                    apkv_in[:, 128 * st:128 * st + 128, 64:128]
                    .rearrange("d p m -> p d m"),
                    proj[:, st, 2560:3072].rearrange("p (d m) -> p d m", m=64))
            nc.gpsimd.collective_compute(
                "AllToAll", mybir.AluOpType.bypass,
                replica_groups=groups, ins=[apkv_in.opt()], outs=[apkv_out.opt()],
            )
            # --- q pair 0 ---
            for ch in (0, 1):
                proj_chunk(ch)
                for st in range(2):
                    nc.gpsimd.dma_start(
                        apq0_in[4 * ch:4 * ch + 4, 128 * st:128 * st + 128, :]
                        .rearrange("d p m -> p d m"),
                        proj[:, st, CH * ch:CH * ch + CH]
                        .rearrange("p (d m) -> p d m", m=128))
            nc.gpsimd.collective_compute(
                "AllToAll", mybir.AluOpType.bypass,
                replica_groups=groups, ins=[apq0_in.opt()], outs=[apq0_out.opt()],
            )
            # --- q pair 1 ---
            for ch in (2, 3):
                proj_chunk(ch)
                for st in range(2):
                    nc.gpsimd.dma_start(
                        apq1_in[4 * (ch - 2):4 * (ch - 2) + 4,
                                128 * st:128 * st + 128, :]
                        .rearrange("d p m -> p d m"),
                        proj[:, st, CH * ch:CH * ch + CH]
                        .rearrange("p (d m) -> p d m", m=128))
            nc.gpsimd.collective_compute(
                "AllToAll", mybir.AluOpType.bypass,
                replica_groups=groups, ins=[apq1_in.opt()], outs=[apq1_out.opt()],
            )

            # ---------------- receiver staging ----------------
            # qT comes straight from XBAR transposing DMAs (128-col payload
            # qualifies for the fast path); k is staged duplicated so one PE
            # transpose per tile yields both kT row-halves
            qT = [pp.tile([128, SEQ], BF, name=f"qT{p}") for p in range(2)]
            kT = pp.tile([128, SEQ], BF, name="kT")
            v_sb = pp.tile([128, KT, 2 * HD], BF, name="v_sb")
            nc.gpsimd.memset(v_sb[:, :, HD:2 * HD], 1.0)

            stage_k2 = pp.tile([128, KT, 2, 64], BF, name="stage_k2")
            for h in range(2):
                nc.sync.dma_start(
                    stage_k2[:, :, h, :],
                    apkv_out[:, :, 0:64].rearrange("s (t p) m -> p (s t) m",
                                                   p=128))
            nc.sync.dma_start(
                v_sb[:, :, 0:HD],
                apkv_out[:, :, 64:128].rearrange("s (t p) m -> p (s t) m", p=128))

            def tk_build(g):       # one packed transpose -> both kT halves
                tk = psp.tile([128, 128], BF, tag="ps", bufs=3, name="tk")
                nc.tensor.transpose(tk[:], stage_k2[:, g, :, :], ident[:])
                nc.vector.tensor_copy(kT[:, 128 * g:128 * g + 128], tk[:])

            def q_transpose_dma(pair, j, eng):
                apq_out = apq0_out if pair == 0 else apq1_out
                eng.dma_start_transpose(
                    qT[pair][:, CH * j:CH * j + CH],
                    apq_out[2 * j:2 * j + 2, :, :]
                    .rearrange("s r m -> (s r) m"))

            # ---------------- attention ----------------
            attnT = pp.tile([128, 2, SEQ], BF, name="attnT")

            def attention(pair, j, interleave=None):
                nkt = 4 * j + 4
                pso0 = psp.tile([2 * HD, CH], F32, tag="ps", bufs=3, name="pso0")
                pso1 = psp.tile([2 * HD, CH], F32, tag="ps", bufs=3, name="pso1")
                qsl = slice(CH * j, CH * j + CH)
                qTt = qT[pair]
                for kt in range(nkt):
                    ks = slice(128 * kt, 128 * kt + 128)
                    t = kt - 4 * j        # >= 0 on the diagonal band
                    c0 = 128 * t if t >= 0 else 0
                    qs = slice(CH * j + c0, CH * j + CH)
                    sp = psp.tile([128, 2, CH], F32, tag="spair", bufs=2, name="sp")
                    nc.tensor.matmul(sp[:, 0, c0:CH], kT[0:64, ks],
                                     qTt[0:64, qs], start=True, stop=True)
                    nc.tensor.matmul(sp[:, 1, c0:CH], kT[64:128, ks],
                                     qTt[64:128, qs], start=True, stop=True)
                    ep = wp.tile([128, 2, CH], BF, tag="exps", bufs=4, name="ep")
                    nc.scalar.activation(ep[:, :, c0:CH], sp[:, :, c0:CH],
                                         mybir.ActivationFunctionType.Exp,
                                         scale=0.125)
                    if t >= 0:
                        nc.vector.tensor_tensor(ep[:, :, c0:c0 + 128],
                                                ep[:, :, c0:c0 + 128],
                                                tri2[:], MUL)
                    nc.tensor.matmul(pso0[:, c0:CH], v_sb[:, kt, :],
                                     ep[:, 0, c0:CH],
                                     start=(kt == 0), stop=(kt == nkt - 1))
                    nc.tensor.matmul(pso1[:, c0:CH], v_sb[:, kt, :],
                                     ep[:, 1, c0:CH],
                                     start=(kt == 0), stop=(kt == nkt - 1))
                    if interleave is not None:
                        interleave(j, kt)
                for h, pso in ((0, pso0), (1, pso1)):
                    bc = wp.tile([64, CH], F32, tag="bcast", bufs=2, name="bc")
                    nc.vector.tensor_copy(bc[:], pso[HD:2 * HD, :])
                    rc = wp.tile([64, CH], F32, tag="rcp", bufs=2, name="rc")
                    nc.vector.reciprocal_approx_fast(out=rc[:], in_=bc[:])
                    nc.vector.tensor_tensor(
                        attnT[64 * h:64 * h + 64, pair, qsl],
                        pso[0:HD, :], rc[:], MUL)

            # ---------------- output projection helpers ----------------
            woA = pp.tile([128, DT // 2, DIM], BF, name="woA")
            woB = pp.tile([128, DT // 2, DIM], BF, name="woB")
            a2a_sb0 = pp.tile([128, NC_CORES, SC], BF, name="a2a_sb0")
            a2a_sb1 = pp.tile([128, NC_CORES, SC], BF, name="a2a_sb1")
            partials = pp.tile([128, 2 * NCH, CH], BF, tag="proj",
                               name="partials")
            evens = [2 * src for src in range(NC_CORES)]
            odds = [2 * src + 1 for src in range(NC_CORES)]
            chunks = [(qt, nch) for qt in range(2) for nch in range(NCH)]

            def op_mm(psf, qt, nsl, g, start, stop):
                w_ap = (woA[:, g, nsl] if g < DT // 2
                        else woB[:, g - DT // 2, nsl])
                a_ap = (a2a_sb0[:, g // 2, 128 * qt:128 * qt + 128] if g % 2 == 0
                        else a2a_sb1[:, g // 2, 128 * qt:128 * qt + 128])
                nc.tensor.matmul(psf[:], a_ap, w_ap, start=start, stop=stop)

            # drip-feed state for the even (pair-0) half of the out-projection
            ev_state = {"psf": None, "n": 0}

            def even_steps(nsteps):
                # emit `nsteps` matmuls of the even-half out-projection,
                # opening/closing psum groups of 8 as needed
                for _ in range(nsteps):
                    n = ev_state["n"]
                    if n >= 64:
                        return
                    i8, i = divmod(n, NC_CORES)
                    qt, nch2 = chunks[i8]
                    if i == 0:
                        ev_state["psf"] = psp.tile([128, CH], F32, tag="psf",
                                                   bufs=1, name="psfE")
                    nsl = slice(CH * nch2, CH * nch2 + CH)
                    op_mm(ev_state["psf"], qt, nsl, evens[i],
                          i == 0, i == NC_CORES - 1)
                    if i == NC_CORES - 1:
                        nc.vector.tensor_copy(partials[:, i8, :],
                                              ev_state["psf"][:])
                    ev_state["n"] = n + 1

            # ---------------- pair-0 attention ----------------
            # kT builds run in the A2A wait window; qT arrives per-chunk via
            # transposing DMAs (scalar queue is free until the first exp)
            for g in range(KT):
                tk_build(g)
            for j in range(NCH):
                q_transpose_dma(0, j, nc.scalar)

            for j in range(NCH):
                if j == 0:
                    for j1 in range(NCH):
                        q_transpose_dma(1, j1, nc.sync)
                attention(0, j)
                nc.gpsimd.dma_start(
                    a2a_in0[2 * j:2 * j + 2, :, :]
                    .rearrange("d p m -> p d m"),
                    attnT[:, 0, CH * j:CH * j + CH]
                    .rearrange("p (d m) -> p d m", m=SC))
                # anchored wo prefetch (the scheduler hoists dep-free DMAs)
                nc.vector.tensor_copy(woA[0:1, 2 * j, 0:1],
                                      attnT[0:1, 0, CH * j:CH * j + 1])
                nc.sync.dma_start(
                    woA[:, 2 * j:2 * j + 2, :],
                    wo[256 * j:256 * j + 256, :].rearrange("(t p) n -> p t n",
                                                           p=128))
                if j >= 2:   # woB too: needed by the interleaved even groups
                    jb = j - 2
                    nc.vector.tensor_copy(woB[0:1, 4 * jb, 0:1],
                                          attnT[0:1, 0, CH * j:CH * j + 1])
                    nc.gpsimd.dma_start(
                        woB[:, 4 * jb:4 * jb + 4, :],
                        wo[1024 + 512 * jb:1024 + 512 * jb + 512, :]
                        .rearrange("(t p) n -> p t n", p=128))
            nc.gpsimd.collective_compute(
                "AllToAll", mybir.AluOpType.bypass,
                replica_groups=groups, ins=[a2a_in0.opt()], outs=[a2a_out0.opt()],
            )
            for half, eng in ((0, nc.sync), (1, nc.scalar)):
                eng.dma_start(
                    a2a_sb0[:, :, 128 * half:128 * half + 128],
                    a2a_out0[:, :, 128 * half:128 * half + 128]
                    .rearrange("s p m -> p s m"))

            # ---------------- pair-1 attention + drip-fed even outproj --------
            # 1 even matmul per k-tile in chunks 2-3 (3 groups land inside
            # pair-1); 5 groups are saved to fill the final-A2A window
            def interleave_p1(j, kt):
                if j >= 2 and ev_state["n"] < 24:
                    even_steps(1)

            for j in range(NCH):
                attention(1, j, interleave=interleave_p1)
                nc.gpsimd.dma_start(
                    a2a_in1[2 * j:2 * j + 2, :, :]
                    .rearrange("d p m -> p d m"),
                    attnT[:, 1, CH * j:CH * j + CH]
                    .rearrange("p (d m) -> p d m", m=SC))

            # ---------------- final A2A + remaining outproj ----------------
            nc.gpsimd.collective_compute(
                "AllToAll", mybir.AluOpType.bypass,
                replica_groups=groups, ins=[a2a_in1.opt()], outs=[a2a_out1.opt()],
            )
            even_steps(64)        # groups 3-7 fill the A2A window
            for half, eng in ((0, nc.sync), (1, nc.scalar)):
                eng.dma_start(
                    a2a_sb1[:, :, 128 * half:128 * half + 128],
                    a2a_out1[:, :, 128 * half:128 * half + 128]
                    .rearrange("s p m -> p s m"))

            for i8, (qt, nch2) in enumerate(chunks):
                psf = psp.tile([128, CH], F32, tag="psf", bufs=1, name="psfO")
                nsl = slice(CH * nch2, CH * nch2 + CH)
                for i, g in enumerate(odds):
                    op_mm(psf, qt, nsl, g, i == 0, i == NC_CORES - 1)
                osb = wp.tile([128, CH], F32, tag="osb", bufs=2, name="osb")
                nc.vector.tensor_tensor(osb[:], psf[:], partials[:, i8, :], ADD)
                eng = nc.sync if i8 % 2 == 0 else nc.scalar
                eng.dma_start(out[128 * qt:128 * qt + 128, nsl], osb[:])

    nc.finalize()
    return nc


def _get_nc():
    if "nc" not in _CACHE:
        _CACHE["nc"] = _build_nc()
    return _CACHE["nc"]


_PERM = np.concatenate([np.arange(0, HD, 2), np.arange(1, HD, 2)])  # de-interleave


def _shard(inputs):
    import ml_dtypes
    x = np.ascontiguousarray(inputs["x"][0].astype(np.float32))          # [S, D]
    wq, wk, wv = (np.asarray(inputs[k]).astype(np.float32) for k in ("wq", "wk", "wv"))
    wo = np.ascontiguousarray(np.asarray(inputs["wo"]).astype(ml_dtypes.bfloat16))
    cos = np.asarray(inputs["freqs_cos"]).astype(np.float32)
    sin = np.asarray(inputs["freqs_sin"]).astype(np.float32)
    # W_all columns: [q-pair0 (8x128) | q-pair1 (8x128) | k (8x64) | v (8x64)],
    # q/k head-dims de-interleaved ([32 evens | 32 odds] per head)
    wq_p = wq.reshape(DIM, 32, HD)[:, :, _PERM].reshape(DIM, 32, HD)
    wk_p = wk.reshape(DIM, 8, HD)[:, :, _PERM]
    q0 = np.concatenate([wq_p[:, 4 * c:4 * c + 2, :].reshape(DIM, 128)
                         for c in range(NC_CORES)], axis=1)
    q1 = np.concatenate([wq_p[:, 4 * c + 2:4 * c + 4, :].reshape(DIM, 128)
                         for c in range(NC_CORES)], axis=1)
    w_all = np.ascontiguousarray(
        np.concatenate([q0, q1, wk_p.reshape(DIM, 512), wv], axis=1)
        .astype(ml_dtypes.bfloat16))
    # triangle mask for the diagonal 128x128 block (keep col >= row)
    tri = (np.arange(128)[None, :] >= np.arange(128)[:, None]).astype(np.float32)
    tri2 = np.ascontiguousarray(
        np.broadcast_to(tri[:, None, :], (128, 2, 128)).astype(ml_dtypes.bfloat16))
    in_maps = []
    for c in range(NC_CORES):
        xc = x[SC * c:SC * (c + 1), :]                    # [256, 2048]
        # xT layout [128 part, DT, SC]: [p, t, m] = xc[m, 128 t + p]
        xTl = np.ascontiguousarray(
            xc.T.reshape(DT, 128, SC).transpose(1, 0, 2).astype(ml_dtypes.bfloat16))
        cs = cos[SC * c:SC * (c + 1), :].reshape(2, 128, 32)
        sn = sin[SC * c:SC * (c + 1), :].reshape(2, 128, 32)
        cos_rep = np.ascontiguousarray(np.broadcast_to(
            cs.transpose(1, 0, 2)[:, :, None, :], (128, 2, 8, 32))
            .astype(ml_dtypes.bfloat16))
        sin_rep = np.ascontiguousarray(np.broadcast_to(
            sn.transpose(1, 0, 2)[:, :, None, :], (128, 2, 8, 32))
            .astype(ml_dtypes.bfloat16))
        in_maps.append({
            "xT": xTl,
            "w_all": w_all,
            "wo": wo,
            "cos_rep": cos_rep,
            "sin_rep": sin_rep,
            "tri2": tri2,
        })
    return in_maps


def kernel(**inputs):
    from concourse.bass_utils import run_bass_kernel_spmd

    nc = _get_nc()
    in_maps = _shard(inputs)
    res = run_bass_kernel_spmd(nc, in_maps, core_ids=list(range(NC_CORES)))
    out = np.concatenate([res.results[c]["out"] for c in range(NC_CORES)], axis=0)
    return out[None].astype(np.float32)


# revision 30
# speedup vs baseline: 1.3293x; 1.0040x over previous
"""Tensor-parallel GQA attention forward for one TRN2 chip (8 NeuronCores).

Strategy (8-way tensor parallel over heads):
  - each core owns 4 q-heads + 1 kv-head (wq/wk/wv column-sharded, host side)
  - x arrives pre-transposed and pre-cast to bf16 from the host (xT layout
    [128, 16, 256]); RoPE tables arrive pre-replicated; the causal triangle
    mask arrives precomputed
  - projections are sequence-sharded: each core projects its own 256 rows
    against all 3072 weight columns; k, v and q-pair-0 go out in a single
    merged AllToAll (minimizes the serial collective chain after the entry
    barrier), q-pair-1 in a second one
  - scores are computed transposed (S^T[k, q]) so exp runs straight out of
    PSUM; softmax denominators come for free as ones-columns in the PV
    matmul; causal masking = skipping k-tiles above the diagonal, a
    column-trapezoid restriction on the 4 diagonal-band tiles, and a
    128-wide triangle multiply on the diagonal block
  - receiver-side kT/qT transposes are staged across the pair-0 attention
    chunks (1-2 per k-tile) so the PE stays dense and the first exp starts
    as early as possible
  - an AllToAll flips head-sharded attnT to sequence-sharded; the output
    projection's pair-0 half is drip-fed into pair-1 attention (2 matmuls
    per k-tile) with three groups reserved to fill the final-AllToAll
    window; the pair-1 half runs after it
  - compute dtype bf16 (fp32 PSUM accumulation), output fp32
"""

import numpy as np

NC_CORES = 8
SEQ = 2048
DIM = 2048
HD = 64            # head dim
SC = SEQ // NC_CORES   # 256: sequence rows per core (proj shard / output shard)
CH = 512           # q-chunk width for attention
NCH = SEQ // CH    # 4
KT = SEQ // 128    # 16 k-tiles
DT = DIM // 128    # 16 d-tiles

_CACHE = {}


def _build_nc():
    import concourse.bass as bass
    import concourse.mybir as mybir
    import concourse.tile as tile
    from concourse import bacc
    from concourse.masks import make_identity

    BF = mybir.dt.bfloat16
    F32 = mybir.dt.float32
    MUL = mybir.AluOpType.mult
    ADD = mybir.AluOpType.add
    SUB = mybir.AluOpType.subtract

    nc = bacc.Bacc("TRN2", target_bir_lowering=False, debug=False,
                   num_devices=NC_CORES)

    # ---- external I/O (per-core shards) ----
    # W_all columns: [q-pair0: 8x128 | q-pair1: 8x128 | k: 8x64 | v: 8x64]
    xT = nc.dram_tensor("xT", [128, DT, SC], BF, kind="ExternalInput")
    w_all = nc.dram_tensor("w_all", [DIM, DIM + 2 * 512], BF, kind="ExternalInput")
    wo = nc.dram_tensor("wo", [DIM, DIM], BF, kind="ExternalInput")
    cos_rep_in = nc.dram_tensor("cos_rep", [128, 2, 8, 32], BF, kind="ExternalInput")
    sin_rep_in = nc.dram_tensor("sin_rep", [128, 2, 8, 32], BF, kind="ExternalInput")
    tri2_in = nc.dram_tensor("tri2", [128, 2, 128], BF, kind="ExternalInput")
    out = nc.dram_tensor("out", [SC, DIM], F32, kind="ExternalOutput")

    groups = [list(range(NC_CORES))]
    WCOLS = DIM + 1024          # 3072

    with tile.TileContext(nc) as tc:
        # DRAM bounce buffers for collectives
        apkv_in, _ = tc.tile([NC_CORES, SC, 128], BF, space=bass.MemorySpace.DRAM,
                             name="apkv_in")
        apkv_out, _ = tc.tile([NC_CORES, SC, 128], BF, space=bass.MemorySpace.DRAM,
                              addr_space="Shared", name="apkv_out")
        apq0_in, _ = tc.tile([NC_CORES, SC, 128], BF, space=bass.MemorySpace.DRAM,
                             name="apq0_in")
        apq0_out, _ = tc.tile([NC_CORES, SC, 128], BF, space=bass.MemorySpace.DRAM,
                              addr_space="Shared", name="apq0_out")
        apq1_in, _ = tc.tile([NC_CORES, SC, 128], BF, space=bass.MemorySpace.DRAM,
                             name="apq1_in")
        apq1_out, _ = tc.tile([NC_CORES, SC, 128], BF, space=bass.MemorySpace.DRAM,
                              addr_space="Shared", name="apq1_out")
        a2a_in0, _ = tc.tile([NC_CORES, 128, SC], BF,
                             space=bass.MemorySpace.DRAM, name="a2a_in0")
        a2a_out0, _ = tc.tile([NC_CORES, 128, SC], BF,
                              space=bass.MemorySpace.DRAM,
                              addr_space="Shared", name="a2a_out0")
        a2a_in1, _ = tc.tile([NC_CORES, 128, SC], BF,
                             space=bass.MemorySpace.DRAM, name="a2a_in1")
        a2a_out1, _ = tc.tile([NC_CORES, 128, SC], BF,
                              space=bass.MemorySpace.DRAM,
                              addr_space="Shared", name="a2a_out1")

        with tc.tile_pool(name="persist", bufs=1) as pp, \
             tc.tile_pool(name="wstream", bufs=2) as wsp, \
             tc.tile_pool(name="work", bufs=2) as wp, \
             tc.tile_pool(name="psum", bufs=2, space="PSUM") as psp:

            # host-prepped xT slice first, split across queues so the first
            # proj matmuls (dt 0-7) can start after the half-loads land
            xTc = pp.tile([128, DT, SC], BF, name="xTc")
            nc.gpsimd.dma_start(xTc[:, 0:8, :], xT[:, 0:8, :])
            nc.scalar.dma_start(xTc[:, 8:16, :], xT[:, 8:16, :])
            ident = pp.tile([128, 128], BF, name="ident")
            make_identity(nc, ident[:])
            cos_rep = pp.tile([128, 2, 8, 32], BF, name="cos_rep")
            sin_rep = pp.tile([128, 2, 8, 32], BF, name="sin_rep")
            nc.scalar.dma_start(cos_rep[:], cos_rep_in[:])
            nc.scalar.dma_start(sin_rep[:], sin_rep_in[:])
            tri2 = pp.tile([128, 2, 128], BF, name="tri2")
            nc.scalar.dma_start(tri2[:], tri2_in[:])

            # ---------------- seq-sharded projections (all heads, own 256 s) ----
            # W chunk order: k, v, q-pair0 first (merged A2A issued earliest),
            # then q-pair1.
            proj = pp.tile([128, 2, WCOLS], BF, name="proj")

            def proj_chunk(ch):
                wt = wsp.tile([128, DT, CH], BF, tag="wt", bufs=3, name="wt")
                for hf in range(2):
                    eng = nc.sync if hf == 0 else nc.scalar
                    eng.dma_start(
                        wt[:, 8 * hf:8 * hf + 8, :],
                        w_all[1024 * hf:1024 * hf + 1024, CH * ch:CH * ch + CH]
                        .rearrange("(t p) m -> p t m", p=128))
                for st in range(2):
                    psq = psp.tile([128, CH], F32, tag="ps", bufs=3, name="psq")
                    for dt in range(DT):
                        nc.tensor.matmul(
                            psq[:], xTc[:, dt, 128 * st:128 * st + 128],
                            wt[:, dt, :],
                            start=(dt == 0), stop=(dt == DT - 1))
                    if ch < 5:   # q and k columns get RoPE (8 head-pairs/chunk)
                        nh = 8
                        pv = psq[:].rearrange("p (h x) -> p h x", x=32)
                        ta = wp.tile([128, 8, 32], F32, tag="ropeA", bufs=2, name="ta")
                        tb = wp.tile([128, 8, 32], F32, tag="ropeB", bufs=2, name="tb")
                        dstv = proj[:, st, CH * ch:CH * ch + CH].rearrange(
                            "p (h x) -> p h x", x=32)
                        crep = cos_rep[:, st, 0:nh, :]
                        srep = sin_rep[:, st, 0:nh, :]
                        qr = pv[:, 0:2 * nh:2, :]
                        qi = pv[:, 1:2 * nh:2, :]
                        nc.vector.tensor_tensor(ta[:, 0:nh, :], qr, crep, MUL)
                        nc.vector.tensor_tensor(tb[:, 0:nh, :], qi, srep, MUL)
                        nc.vector.tensor_tensor(dstv[:, 0:2 * nh:2, :],
                                                ta[:, 0:nh, :], tb[:, 0:nh, :], SUB)
                        nc.vector.tensor_tensor(ta[:, 0:nh, :], qr, srep, MUL)
                        nc.vector.tensor_tensor(tb[:, 0:nh, :], qi, crep, MUL)
                        nc.vector.tensor_tensor(dstv[:, 1:2 * nh:2, :],
                                                ta[:, 0:nh, :], tb[:, 0:nh, :], ADD)
                    else:
                        nc.vector.tensor_copy(proj[:, st, CH * ch:CH * ch + CH],
                                              psq[:])

            # --- kv -> first A2A (its wire time overlaps the q projections) ---
            proj_chunk(4)
            for st in range(2):
                nc.gpsimd.dma_start(
                    apkv_in[:, 128 * st:128 * st + 128, 0:64]
                    .rearrange("d p m -> p d m"),
                    proj[:, st, 2048:2560].rearrange("p (d m) -> p d m", m=64))
            proj_chunk(5)
            for st in range(2):
                nc.gpsimd.dma_start(
                    apkv_in[:, 128 * st:128 * st + 128, 64:128]
                    .rearrange("d p m -> p d m"),
                    proj[:, st, 2560:3072].rearrange("p (d m) -> p d m", m=64))
            nc.gpsimd.collective_compute(
                "AllToAll", mybir.AluOpType.bypass,
                replica_groups=groups, ins=[apkv_in.opt()], outs=[apkv_out.opt()],
            )
            # --- q pair 0 ---
            for ch in (0, 1):
                proj_chunk(ch)
                for st in range(2):
                    nc.gpsimd.dma_start(
                        apq0_in[4 * ch:4 * ch + 4, 128 * st:128 * st + 128, :]
                        .rearrange("d p m -> p d m"),
                        proj[:, st, CH * ch:CH * ch + CH]
                        .rearrange("p (d m) -> p d m", m=128))
            nc.gpsimd.collective_compute(
                "AllToAll", mybir.AluOpType.bypass,
                replica_groups=groups, ins=[apq0_in.opt()], outs=[apq0_out.opt()],
            )
            # --- q pair 1 ---
            for ch in (2, 3):
                proj_chunk(ch)
                for st in range(2):
                    nc.gpsimd.dma_start(
                        apq1_in[4 * (ch - 2):4 * (ch - 2) + 4,
                                128 * st:128 * st + 128, :]
                        .rearrange("d p m -> p d m"),
                        proj[:, st, CH * ch:CH * ch + CH]
                        .rearrange("p (d m) -> p d m", m=128))
            nc.gpsimd.collective_compute(
                "AllToAll", mybir.AluOpType.bypass,
                replica_groups=groups, ins=[apq1_in.opt()], outs=[apq1_out.opt()],
            )

            # ---------------- receiver staging ----------------
            # qT comes straight from XBAR transposing DMAs (128-col payload
            # qualifies for the fast path); k is staged duplicated so one PE
            # transpose per tile yields both kT row-halves
            qT = [pp.tile([128, SEQ], BF, name=f"qT{p}") for p in range(2)]
            kT = pp.tile([128, SEQ], BF, name="kT")
            v_sb = pp.tile([128, KT, 2 * HD], BF, name="v_sb")
            nc.gpsimd.memset(v_sb[:, :, HD:2 * HD], 1.0)

            stage_k2 = pp.tile([128, KT, 2, 64], BF, name="stage_k2")
            for h in range(2):
                nc.sync.dma_start(
                    stage_k2[:, :, h, :],
                    apkv_out[:, :, 0:64].rearrange("s (t p) m -> p (s t) m",
                                                   p=128))
            nc.sync.dma_start(
                v_sb[:, :, 0:HD],
                apkv_out[:, :, 64:128].rearrange("s (t p) m -> p (s t) m", p=128))

            def tk_build(g):       # one packed transpose -> both kT halves
                tk = psp.tile([128, 128], BF, tag="ps", bufs=3, name="tk")
                nc.tensor.transpose(tk[:], stage_k2[:, g, :, :], ident[:])
                nc.vector.tensor_copy(kT[:, 128 * g:128 * g + 128], tk[:])

            def q_transpose_dma(pair, j, eng):
                apq_out = apq0_out if pair == 0 else apq1_out
                eng.dma_start_transpose(
                    qT[pair][:, CH * j:CH * j + CH],
                    apq_out[2 * j:2 * j + 2, :, :]
                    .rearrange("s r m -> (s r) m"))

            # ---------------- attention ----------------
            attnT = pp.tile([128, 2, SEQ], BF, name="attnT")

            def attention(pair, j, interleave=None):
                nkt = 4 * j + 4
                pso0 = psp.tile([2 * HD, CH], F32, tag="ps", bufs=3, name="pso0")
                pso1 = psp.tile([2 * HD, CH], F32, tag="ps", bufs=3, name="pso1")
                qsl = slice(CH * j, CH * j + CH)
                qTt = qT[pair]
                for kt in range(nkt):
                    ks = slice(128 * kt, 128 * kt + 128)
                    t = kt - 4 * j        # >= 0 on the diagonal band
                    c0 = 128 * t if t >= 0 else 0
                    qs = slice(CH * j + c0, CH * j + CH)
                    sp = psp.tile([128, 2, CH], F32, tag="spair", bufs=2, name="sp")
                    nc.tensor.matmul(sp[:, 0, c0:CH], kT[0:64, ks],
                                     qTt[0:64, qs], start=True, stop=True)
                    nc.tensor.matmul(sp[:, 1, c0:CH], kT[64:128, ks],
                                     qTt[64:128, qs], start=True, stop=True)
                    ep = wp.tile([128, 2, CH], BF, tag="exps", bufs=4, name="ep")
                    nc.scalar.activation(ep[:, :, c0:CH], sp[:, :, c0:CH],
                                         mybir.ActivationFunctionType.Exp,
                                         scale=0.125)
                    if t >= 0:
                        nc.vector.tensor_tensor(ep[:, :, c0:c0 + 128],
                                                ep[:, :, c0:c0 + 128],
                                                tri2[:], MUL)
                    nc.tensor.matmul(pso0[:, c0:CH], v_sb[:, kt, :],
                                     ep[:, 0, c0:CH],
                                     start=(kt == 0), stop=(kt == nkt - 1))
                    nc.tensor.matmul(pso1[:, c0:CH], v_sb[:, kt, :],
                                     ep[:, 1, c0:CH],
                                     start=(kt == 0), stop=(kt == nkt - 1))
                    if interleave is not None:
                        interleave(j, kt)
                for h, pso in ((0, pso0), (1, pso1)):
                    bc = wp.tile([64, CH], F32, tag="bcast", bufs=2, name="bc")
                    nc.vector.tensor_copy(bc[:], pso[HD:2 * HD, :])
                    rc = wp.tile([64, CH], F32, tag="rcp", bufs=2, name="rc")
                    nc.vector.reciprocal_approx_fast(out=rc[:], in_=bc[:])
                    nc.vector.tensor_tensor(
                        attnT[64 * h:64 * h + 64, pair, qsl],
                        pso[0:HD, :], rc[:], MUL)

            # ---------------- output projection helpers ----------------
            woA = pp.tile([128, DT // 2, DIM], BF, name="woA")
            woB = pp.tile([128, DT // 2, DIM], BF, name="woB")
            a2a_sb0 = pp.tile([128, NC_CORES, SC], BF, name="a2a_sb0")
            a2a_sb1 = pp.tile([128, NC_CORES, SC], BF, name="a2a_sb1")
            partials = pp.tile([128, 2 * NCH, CH], BF, tag="proj",
                               name="partials")
            evens = [2 * src for src in range(NC_CORES)]
            odds = [2 * src + 1 for src in range(NC_CORES)]
            chunks = [(qt, nch) for qt in range(2) for nch in range(NCH)]

            def op_mm(psf, qt, nsl, g, start, stop):
                w_ap = (woA[:, g, nsl] if g < DT // 2
                        else woB[:, g - DT // 2, nsl])
                a_ap = (a2a_sb0[:, g // 2, 128 * qt:128 * qt + 128] if g % 2 == 0
                        else a2a_sb1[:, g // 2, 128 * qt:128 * qt + 128])
                nc.tensor.matmul(psf[:], a_ap, w_ap, start=start, stop=stop)

            # drip-feed state for the even (pair-0) half of the out-projection
            ev_state = {"psf": None, "n": 0}

            def even_steps(nsteps):
                # emit `nsteps` matmuls of the even-half out-projection,
                # opening/closing psum groups of 8 as needed
                for _ in range(nsteps):
                    n = ev_state["n"]
                    if n >= 64:
                        return
                    i8, i = divmod(n, NC_CORES)
                    qt, nch2 = chunks[i8]
                    if i == 0:
                        ev_state["psf"] = psp.tile([128, CH], F32, tag="psf",
                                                   bufs=1, name="psfE")
                    nsl = slice(CH * nch2, CH * nch2 + CH)
                    op_mm(ev_state["psf"], qt, nsl, evens[i],
                          i == 0, i == NC_CORES - 1)
                    if i == NC_CORES - 1:
                        nc.vector.tensor_copy(partials[:, i8, :],
                                              ev_state["psf"][:])
                    ev_state["n"] = n + 1

            # ---------------- pair-0 attention ----------------
            # kT builds run in the A2A wait window; qT arrives per-chunk via
            # transposing DMAs (scalar queue is free until the first exp)
            for g in range(KT):
                tk_build(g)
            for j in range(NCH):
                q_transpose_dma(0, j, nc.scalar)

            for j in range(NCH):
                if j == 0:
                    for j1 in range(NCH):
                        q_transpose_dma(1, j1, nc.sync)
                attention(0, j)
                nc.gpsimd.dma_start(
                    a2a_in0[2 * j:2 * j + 2, :, :]
                    .rearrange("d p m -> p d m"),
                    attnT[:, 0, CH * j:CH * j + CH]
                    .rearrange("p (d m) -> p d m", m=SC))
                # anchored wo prefetch (the scheduler hoists dep-free DMAs)
                nc.vector.tensor_copy(woA[0:1, 2 * j, 0:1],
                                      attnT[0:1, 0, CH * j:CH * j + 1])
                nc.sync.dma_start(
                    woA[:, 2 * j:2 * j + 2, :],
                    wo[256 * j:256 * j + 256, :].rearrange("(t p) n -> p t n",
                                                           p=128))
                if j >= 2:   # woB too: needed by the interleaved even groups
                    jb = j - 2
                    nc.vector.tensor_copy(woB[0:1, 4 * jb, 0:1],
                                          attnT[0:1, 0, CH * j:CH * j + 1])
                    nc.gpsimd.dma_start(
                        woB[:, 4 * jb:4 * jb + 4, :],
                        wo[1024 + 512 * jb:1024 + 512 * jb + 512, :]
                        .rearrange("(t p) n -> p t n", p=128))
            nc.gpsimd.collective_compute(
                "AllToAll", mybir.AluOpType.bypass,
                replica_groups=groups, ins=[a2a_in0.opt()], outs=[a2a_out0.opt()],
            )
            for half, eng in ((0, nc.sync), (1, nc.scalar)):
                eng.dma_start(
                    a2a_sb0[:, :, 128 * half:128 * half + 128],
                    a2a_out0[:, :, 128 * half:128 * half + 128]
                    .rearrange("s p m -> p s m"))

            # ---------------- pair-1 attention + drip-fed even outproj --------
            # 1 even matmul per k-tile in chunk 2 (1 group lands inside
            # pair-1 to keep density); 7 groups are saved so the PE stays
            # busy through the final-A2A + sb1-load window (no re-throttle)
            def interleave_p1(j, kt):
                if j >= 2 and ev_state["n"] < 8:
                    even_steps(1)

            for j in range(NCH):
                attention(1, j, interleave=interleave_p1)
                nc.gpsimd.dma_start(
                    a2a_in1[2 * j:2 * j + 2, :, :]
                    .rearrange("d p m -> p d m"),
                    attnT[:, 1, CH * j:CH * j + CH]
                    .rearrange("p (d m) -> p d m", m=SC))

            # ---------------- final A2A + remaining outproj ----------------
            nc.gpsimd.collective_compute(
                "AllToAll", mybir.AluOpType.bypass,
                replica_groups=groups, ins=[a2a_in1.opt()], outs=[a2a_out1.opt()],
            )
            even_steps(64)        # groups 3-7 fill the A2A window
            for half, eng in ((0, nc.sync), (1, nc.scalar)):
                eng.dma_start(
                    a2a_sb1[:, :, 128 * half:128 * half + 128],
                    a2a_out1[:, :, 128 * half:128 * half + 128]
                    .rearrange("s p m -> p s m"))

            for i8, (qt, nch2) in enumerate(chunks):
                psf = psp.tile([128, CH], F32, tag="psf", bufs=1, name="psfO")
                nsl = slice(CH * nch2, CH * nch2 + CH)
                for i, g in enumerate(odds):
                    op_mm(psf, qt, nsl, g, i == 0, i == NC_CORES - 1)
                osb = wp.tile([128, CH], F32, tag="osb", bufs=2, name="osb")
                nc.vector.tensor_tensor(osb[:], psf[:], partials[:, i8, :], ADD)
                eng = nc.sync if i8 % 2 == 0 else nc.scalar
                eng.dma_start(out[128 * qt:128 * qt + 128, nsl], osb[:])

    nc.finalize()
    return nc


def _get_nc():
    if "nc" not in _CACHE:
        _CACHE["nc"] = _build_nc()
    return _CACHE["nc"]


_PERM = np.concatenate([np.arange(0, HD, 2), np.arange(1, HD, 2)])  # de-interleave


def _shard(inputs):
    import ml_dtypes
    x = np.ascontiguousarray(inputs["x"][0].astype(np.float32))          # [S, D]
    wq, wk, wv = (np.asarray(inputs[k]).astype(np.float32) for k in ("wq", "wk", "wv"))
    wo = np.ascontiguousarray(np.asarray(inputs["wo"]).astype(ml_dtypes.bfloat16))
    cos = np.asarray(inputs["freqs_cos"]).astype(np.float32)
    sin = np.asarray(inputs["freqs_sin"]).astype(np.float32)
    # W_all columns: [q-pair0 (8x128) | q-pair1 (8x128) | k (8x64) | v (8x64)],
    # q/k head-dims de-interleaved ([32 evens | 32 odds] per head)
    wq_p = wq.reshape(DIM, 32, HD)[:, :, _PERM].reshape(DIM, 32, HD)
    wk_p = wk.reshape(DIM, 8, HD)[:, :, _PERM]
    q0 = np.concatenate([wq_p[:, 4 * c:4 * c + 2, :].reshape(DIM, 128)
                         for c in range(NC_CORES)], axis=1)
    q1 = np.concatenate([wq_p[:, 4 * c + 2:4 * c + 4, :].reshape(DIM, 128)
                         for c in range(NC_CORES)], axis=1)
    w_all = np.ascontiguousarray(
        np.concatenate([q0, q1, wk_p.reshape(DIM, 512), wv], axis=1)
        .astype(ml_dtypes.bfloat16))
    # triangle mask for the diagonal 128x128 block (keep col >= row)
    tri = (np.arange(128)[None, :] >= np.arange(128)[:, None]).astype(np.float32)
    tri2 = np.ascontiguousarray(
        np.broadcast_to(tri[:, None, :], (128, 2, 128)).astype(ml_dtypes.bfloat16))
    in_maps = []
    for c in range(NC_CORES):
        xc = x[SC * c:SC * (c + 1), :]                    # [256, 2048]
        # xT layout [128 part, DT, SC]: [p, t, m] = xc[m, 128 t + p]
        xTl = np.ascontiguousarray(
            xc.T.reshape(DT, 128, SC).transpose(1, 0, 2).astype(ml_dtypes.bfloat16))
        cs = cos[SC * c:SC * (c + 1), :].reshape(2, 128, 32)
        sn = sin[SC * c:SC * (c + 1), :].reshape(2, 128, 32)
        cos_rep = np.ascontiguousarray(np.broadcast_to(
            cs.transpose(1, 0, 2)[:, :, None, :], (128, 2, 8, 32))
            .astype(ml_dtypes.bfloat16))
        sin_rep = np.ascontiguousarray(np.broadcast_to(
            sn.transpose(1, 0, 2)[:, :, None, :], (128, 2, 8, 32))
            .astype(ml_dtypes.bfloat16))
        in_maps.append({
            "xT": xTl,
            "w_all": w_all,
            "wo": wo,
            "cos_rep": cos_rep,
            "sin_rep": sin_rep,
            "tri2": tri2,
        })
    return in_maps


def kernel(**inputs):
    from concourse.bass_utils import run_bass_kernel_spmd

    nc = _get_nc()
    in_maps = _shard(inputs)
    res = run_bass_kernel_spmd(nc, in_maps, core_ids=list(range(NC_CORES)))
    out = np.concatenate([res.results[c]["out"] for c in range(NC_CORES)], axis=0)
    return out[None].astype(np.float32)
